# revision 5
# baseline (speedup 1.0000x reference)
"""GAU denoising transformer forward pass on 8 Trainium2 NeuronCores.

Data-parallel over batch (B=16 -> 2 images per core); identical NEFF per
core. Residual stream hT kept fp32 in SBUF, transposed (H on partitions x
512 tokens). Big GEMMs run fp8-e4m3 with DoubleRow perf mode (two K=128
chunks contracted per instruction, ~1.8x bf16 MM throughput):

  - u/v/q/k projections: wuv8 = e4m3(64 * Wuv * gnorm) streamed from HBM;
    u/v read h8n = e4m3(h/rms), q/k read h8raw = e4m3(h) (so their GEMMs
    can start before the rms stats finish; the 1/rms lands in the rope
    multiplies). The 1/64 descale folds into the silu input scale.
  - attention is computed TRANSPOSED (scores [m,l] via k-stationary
    matmuls) so softmax sums reduce over partitions with a ones-matmul,
    exp stays unnormalized in fp8, attn@v directly produces oT (feature
    on partitions), and no PE transposes are needed. The 1/sum(exp)
    normalization is computed as exp(-ln(sum)) on [1,512] rows (scalar
    engine, natural_log_exp table set) and folded in after the
    out-projection.
  - out-projection stays bf16 (fp8 there pushes rel-err past the budget).

ACT table sets: per layer exactly two loads (natural_log_exp <-> silu);
squares and fp8 casts run on gpsimd so the set switches hide mid-layer.
rms: hsq8 = e4m3(2*h^2) (gpsimd), sumsq via ones8 DoubleRow matmuls,
1/rms = exp(-0.5*ln(ss) + 0.5*ln(2H)) on a [1,512] row + partition
broadcast.
"""

import sys

for _p in ("/opt/trn_rl_repo",):
    if _p not in sys.path:
        sys.path.append(_p)

import numpy as np
import ml_dtypes

BF = ml_dtypes.bfloat16
F8 = ml_dtypes.float8_e4m3

IMG = 128
P = 8
H = 768
E = 1536
KD = 128          # key size
L = 256           # patches per image
PD = 192          # patch dim
NL = 24
B = 16
NCORES = 8
TOK = 512         # tokens per core (2 images x 256)
HC = H // 128     # 6 h-chunks
EC = E // 128     # 12 e-chunks
SW = 64.0         # fp8 weight scale
WUV_W = 2 * E + 2 * KD    # permuted wuv width: u | q | k | v
QO = E                    # q col offset
KO = E + KD
V0 = E + 2 * KD


def _build(nl=NL, repeat=1):
    """Build + compile the Bass module. Returns nc."""
    import concourse.tile as tile
    from concourse import bacc, mybir

    F32 = mybir.dt.float32
    BF16 = mybir.dt.bfloat16
    FP8 = mybir.dt.float8e4
    AF = mybir.ActivationFunctionType
    DR = mybir.MatmulPerfMode.DoubleRow
    MUL = mybir.AluOpType.mult

    nc = bacc.Bacc("TRN2", target_bir_lowering=False, debug=False,
                   num_devices=NCORES)

    d_xpt = nc.dram_tensor("xpt", [128, 2, TOK], BF16, kind="ExternalInput")
    d_temb = nc.dram_tensor("temb", [128, HC, 2], F32, kind="ExternalInput")
    d_pw = nc.dram_tensor("pw", [128, 2, H], BF16, kind="ExternalInput")
    d_wuv = nc.dram_tensor("wuv", [nl, 128, HC, WUV_W], FP8,
                           kind="ExternalInput")
    d_wout = nc.dram_tensor("wout", [nl, 128, EC, H], BF16,
                            kind="ExternalInput")
    d_upw = nc.dram_tensor("upw", [128, HC, PD], BF16, kind="ExternalInput")
    d_sperm = nc.dram_tensor("sperm", [128, 128], BF16, kind="ExternalInput")
    d_cq = nc.dram_tensor("cq", [128, TOK], BF16, kind="ExternalInput")
    d_sq = nc.dram_tensor("sq", [128, TOK], BF16, kind="ExternalInput")
    d_ck = nc.dram_tensor("ck", [128, TOK], BF16, kind="ExternalInput")
    d_sk = nc.dram_tensor("sk", [128, TOK], BF16, kind="ExternalInput")
    d_out = nc.dram_tensor("outt", [PD, TOK], F32, kind="ExternalOutput")

    from contextlib import ExitStack

    with tile.TileContext(nc) as tc, ExitStack() as ctx:
        pers = ctx.enter_context(tc.tile_pool(name="pers", bufs=1))
        wuvp = ctx.enter_context(tc.tile_pool(name="wuvp", bufs=2))
        woutp = ctx.enter_context(tc.tile_pool(name="woutp", bufs=2))
        rtmp = ctx.enter_context(tc.tile_pool(name="rtmp", bufs=2))
        rows = ctx.enter_context(tc.tile_pool(name="rows", bufs=2))

        psum = ctx.enter_context(tc.tile_pool(name="psum", bufs=1, space="PSUM"))

        # ---- persistent state + constants ----
        hT = [pers.tile([128, TOK], F32, name=f"hT{j}", tag=f"hT{j}")
              for j in range(HC)]
        h8raw = pers.tile([128, HC, TOK], FP8)
        h8n = pers.tile([128, HC, TOK], FP8)
        hsq8 = pers.tile([128, HC, TOK], FP8)
        uT = [pers.tile([128, TOK], BF16, name=f"uT{e}", tag=f"uT{e}")
              for e in range(EC)]
        ogT = [pers.tile([128, TOK], BF16, name=f"ogT{e}", tag=f"ogT{e}")
               for e in range(EC)]
        vg8 = pers.tile([128, 4, E], FP8)
        exp8 = [pers.tile([128, 2, 256], FP8, name=f"exp8_{i}", tag=f"exp8_{i}")
                for i in range(2)]
        qp = pers.tile([128, TOK], BF16)
        kp = pers.tile([128, TOK], BF16)
        qsb = pers.tile([128, TOK], BF16)
        ksb = pers.tile([128, TOK], BF16)
        cq = pers.tile([128, TOK], BF16)
        sq = pers.tile([128, TOK], BF16)
        ck = pers.tile([128, TOK], BF16)
        sk = pers.tile([128, TOK], BF16)
        rb = pers.tile([128, TOK], F32)     # 1/rms broadcast
        rcb = pers.tile([128, TOK], F32)    # 1/sum(exp) broadcast
        temb = pers.tile([128, HC, 2], F32)
        xpt = pers.tile([128, 2, TOK], BF16)
        pw = pers.tile([128, 2, H], BF16)
        upw = pers.tile([128, HC, PD], BF16)
        sperm = pers.tile([128, 128], BF16)
        ones8 = pers.tile([128, 2, 16], FP8)
        twos8 = pers.tile([128, 2, 16], FP8)
        brms = pers.tile([1, 1], F32)       # 0.5*ln(2H)
        bln4 = pers.tile([128, 1], F32)     # -ln(4)

        nc.sync.dma_start(cq, d_cq.ap())
        nc.sync.dma_start(sq, d_sq.ap())
        nc.sync.dma_start(ck, d_ck.ap())
        nc.sync.dma_start(sk, d_sk.ap())
        nc.sync.dma_start(temb, d_temb.ap())
        nc.sync.dma_start(xpt, d_xpt.ap())
        nc.sync.dma_start(pw, d_pw.ap())
        nc.sync.dma_start(upw, d_upw.ap())
        nc.sync.dma_start(sperm, d_sperm.ap())
        nc.vector.memset(ones8, 1.0)
        nc.vector.memset(twos8, 2.0)
        nc.vector.memset(brms, 0.5 * float(np.log(2.0 * H)))
        nc.vector.memset(bln4, float(-np.log(4.0)))

        def tail_chunk(hp, first):
            """After hT[hp] is final for this round: fp8 copies + squares on
            gpsimd, sumsq pair-matmuls into the `prow` psum row."""
            nc.gpsimd.tensor_copy(h8raw[:, hp, :], hT[hp])
            nc.gpsimd.tensor_mul(hsq8[:, hp, :], h8raw[:, hp, :],
                                 h8raw[:, hp, :])
            if hp % 2 == 1:
                t = hp // 2
                nc.tensor.matmul(tail_chunk.ss, twos8[:, :, 0:1],
                                 hsq8[:, hp - 1:hp + 1, :],
                                 start=(t == 0), stop=(t == 2),
                                 perf_mode=DR)

        def new_ss(name):
            tail_chunk.ss = psum.tile([1, TOK], F32, tag="prow",
                                      name=f"ss_{name}")

        def stats(name):
            """rb = 1/rms from the `prow` sumsq row; then h8n = e4(h*rb)."""
            lnr = rows.tile([1, TOK], F32, tag="lnr", name=f"lnr_{name}")
            nc.scalar.activation(lnr, tail_chunk.ss, AF.Ln)
            rbr = rows.tile([1, TOK], F32, tag="rbr", name=f"rbr_{name}")
            nc.scalar.activation(rbr, lnr, AF.Exp, scale=-0.5,
                                 bias=brms[:, 0:1])
            nc.gpsimd.partition_broadcast(rb, rbr)
            for j in range(HC):
                nc.vector.tensor_mul(h8n[:, j, :], hT[j], rb)

        # ---- patchify: hT = patch_W.T @ xp.T + temb ----
        for j in range(HC):
            ps = psum.tile([128, TOK], F32, tag=("pa", "pb")[j % 2])
            for c in range(2):
                nc.tensor.matmul(ps, pw[:, c, j * 128:(j + 1) * 128],
                                 xpt[:, c, :], start=(c == 0), stop=(c == 1))
            for i in range(2):
                nc.vector.tensor_scalar_add(
                    hT[j][:, i * 256:(i + 1) * 256],
                    ps[:, i * 256:(i + 1) * 256],
                    temb[:, j, i:i + 1])
        new_ss("init")
        for j in range(HC):
            tail_chunk(j, first=True)

        for lrep in range(nl * repeat):
            li = lrep % nl
            wuv = wuvp.tile([128, HC, WUV_W], FP8, tag="wuv")
            nc.sync.dma_start(wuv, d_wuv.ap()[li])
            wout = woutp.tile([128, EC, H], BF16, tag="wout")
            nc.sync.dma_start(wout, d_wout.ap()[li])

            # ---- [ln_exp phase] rms stats -> rb, h8n ----
            stats(f"l{lrep}")

            # ---- q/k col-tiles on raw h8 (DoubleRow) + swap via perm ----
            q_ps = psum.tile([128, TOK], F32, tag="pq")
            k_ps = psum.tile([128, TOK], F32, tag="pk")
            for ps, c0 in ((q_ps, QO), (k_ps, KO)):
                for t in range(3):
                    nc.tensor.matmul(ps, wuv[:, 2 * t:2 * t + 2, c0:c0 + 128],
                                     h8raw[:, 2 * t:2 * t + 2, :],
                                     start=(t == 0), stop=(t == 2),
                                     perf_mode=DR)
            nc.scalar.copy(qsb, q_ps)
            nc.scalar.copy(ksb, k_ps)
            qs_ps = psum.tile([128, TOK], F32, tag="pqs")
            nc.tensor.matmul(qs_ps, sperm, qsb, start=True, stop=True)
            ks_ps = psum.tile([128, TOK], F32, tag="pks")
            nc.tensor.matmul(ks_ps, sperm, ksb, start=True, stop=True)

            # ---- rope (x 1/rms): q' = (Q*cos + Qswap*sins) * rb ----
            m1 = rtmp.tile([128, TOK], F32, tag="m1")
            m2 = rtmp.tile([128, TOK], F32, tag="m2")
            nc.vector.tensor_mul(m1, q_ps, cq)
            nc.vector.tensor_mul(m2, qs_ps, sq)
            nc.vector.tensor_add(m1, m1, m2)
            nc.vector.tensor_mul(qp, m1, rb)
            m3 = rtmp.tile([128, TOK], F32, tag="m1")
            m4 = rtmp.tile([128, TOK], F32, tag="m2")
            nc.vector.tensor_mul(m3, k_ps, ck)
            nc.vector.tensor_mul(m4, ks_ps, sk)
            nc.vector.tensor_add(m3, m3, m4)
            nc.vector.tensor_mul(kp, m3, rb)

            # ---- transposed scores + exp (fp8, unnormalized) ----
            for i in range(2):
                scp = psum.tile([128, 2, 256], F32, tag=("pa", "pb")[i])
                for mc in range(2):
                    nc.tensor.matmul(scp[:, mc, :],
                                     kp[:, i * 256 + mc * 128:
                                        i * 256 + (mc + 1) * 128],
                                     qp[:, i * 256:(i + 1) * 256],
                                     start=True, stop=True)
                nc.scalar.activation(exp8[i], scp, AF.Exp,
                                     scale=1.0 / (SW * SW),
                                     bias=bln4[:, 0:1])

            # ---- softmax sums over m (partitions) -> 1/sum row -> rcb ----
            srow = psum.tile([1, TOK], F32, tag="prow", name=f"sr_{lrep}")
            for i in range(2):
                nc.tensor.matmul(srow[:, i * 256:(i + 1) * 256],
                                 ones8[:, :, 0:1], exp8[i],
                                 start=True, stop=True, perf_mode=DR)
            lns = rows.tile([1, TOK], F32, tag="lns", name=f"lns_{lrep}")
            nc.scalar.activation(lns, srow, AF.Ln)
            rcr = rows.tile([1, TOK], F32, tag="rcr", name=f"rcr_{lrep}")
            nc.scalar.activation(rcr, lns, AF.Exp, scale=-1.0)
            nc.gpsimd.partition_broadcast(rcb, rcr)

            # ---- [silu phase] v natural (h8n stationary, DoubleRow) ----
            vtag = ["pa", "pb", "pc"]
            for tk in range(4):
                for ns in range(3):
                    ps = psum.tile([128, 512], F32,
                                   tag=vtag[(tk * 3 + ns) % 3])
                    for t in range(3):
                        nc.tensor.matmul(
                            ps,
                            h8n[:, 2 * t:2 * t + 2, tk * 128:(tk + 1) * 128],
                            wuv[:, 2 * t:2 * t + 2,
                                V0 + ns * 512:V0 + (ns + 1) * 512],
                            start=(t == 0), stop=(t == 2), perf_mode=DR)
                    nc.scalar.activation(vg8[:, tk, ns * 512:(ns + 1) * 512],
                                         ps, AF.Silu, scale=1.0 / SW)

            # ---- u col-tiles (weight-stationary, DoubleRow) ----
            for ct in range(EC):
                ps = psum.tile([128, TOK], F32, tag=vtag[ct % 3])
                for t in range(3):
                    nc.tensor.matmul(ps,
                                     wuv[:, 2 * t:2 * t + 2,
                                         ct * 128:(ct + 1) * 128],
                                     h8n[:, 2 * t:2 * t + 2, :],
                                     start=(t == 0), stop=(t == 2),
                                     perf_mode=DR)
                nc.scalar.activation(uT[ct], ps, AF.Silu, scale=1.0 / SW)

            # ---- oT = (exp @ v).T via v-stationary DoubleRow; gate ----
            for e in range(EC):
                ops = psum.tile([128, TOK], F32, tag=vtag[e % 3])
                for i in range(2):
                    nc.tensor.matmul(ops[:, i * 256:(i + 1) * 256],
                                     vg8[:, 2 * i:2 * i + 2,
                                         e * 128:(e + 1) * 128],
                                     exp8[i], start=True, stop=True,
                                     perf_mode=DR)
                nc.vector.tensor_mul(ogT[e], uT[e], ops)

            # ---- out-projection (bf16) + attn-normalize + residual ----
            new_ss(f"l{lrep}")
            for hp in range(HC):
                dps = psum.tile([128, TOK], F32, tag=vtag[hp % 3])
                for e in range(EC):
                    nc.tensor.matmul(dps, wout[:, e, hp * 128:(hp + 1) * 128],
                                     ogT[e], start=(e == 0), stop=(e == EC - 1))
                tmp = rtmp.tile([128, TOK], F32, tag="rtm")
                nc.vector.tensor_mul(tmp, dps, rcb)
                nc.vector.tensor_add(hT[hp], hT[hp], tmp)
                tail_chunk(hp, first=False)

        # ---- final norm + unpatch (fnorm_w folded into upw on host) ----
        lnr = rows.tile([1, TOK], F32, tag="lnr", name="lnr_fin")
        nc.scalar.activation(lnr, tail_chunk.ss, AF.Ln)
        rbr = rows.tile([1, TOK], F32, tag="rbr", name="rbr_fin")
        nc.scalar.activation(rbr, lnr, AF.Exp, scale=-0.5, bias=brms[:, 0:1])
        nc.gpsimd.partition_broadcast(rb, rbr)
        hfin = uT   # reuse dead uT tiles as bf16 normalized h
        for j in range(HC):
            nc.vector.tensor_mul(hfin[j], hT[j], rb)
        for mchunk, msz in ((0, 128), (1, 64)):
            ps = psum.tile([128, TOK], F32, tag=("pa", "pb")[mchunk])
            for j in range(HC):
                nc.tensor.matmul(ps[:msz, :],
                                 upw[:, j, mchunk * 128:mchunk * 128 + msz],
                                 hfin[j], start=(j == 0), stop=(j == HC - 1))
            osb = rtmp.tile([128, TOK], F32, tag="m1")
            nc.vector.tensor_copy(osb[:msz, :], ps[:msz, :])
            nc.sync.dma_start(d_out.ap()[mchunk * 128:mchunk * 128 + msz, :],
                              osb[:msz, :])

    nc.compile()
    return nc


_BUILD_CACHE = {}


def _get_nc(nl=NL, repeat=1):
    key = (nl, repeat)
    if key not in _BUILD_CACHE:
        _BUILD_CACHE[key] = _build(nl, repeat)
    return _BUILD_CACHE[key]


def _rope_tables():
    pos = np.arange(L)

    def sinemb(p, dim=64, base=1000.0):
        half = dim // 2
        freqs = np.exp(np.arange(half, dtype=np.float32)
                       * np.float32(-np.log(base) / (half - 1)))
        ang = p[:, None].astype(np.float32) * freqs[None, :]
        return np.concatenate([np.sin(ang), np.cos(ang)], axis=-1)

    w = IMG // P
    pe = np.concatenate([sinemb(pos // w), sinemb(pos % w)],
                        axis=-1).astype(np.float32)      # (256, 128)
    sinv = pe[:, :64].T                                  # (64, 256)
    cosv = pe[:, 64:].T
    COS = np.concatenate([cosv, cosv], axis=0)           # (128, 256)
    SINS = np.concatenate([-sinv, sinv], axis=0)
    COS2 = np.tile(COS, (1, 2))                          # (128, 512)
    SINS2 = np.tile(SINS, (1, 2))
    scale = np.float32(KD ** -0.5)
    return (np.ascontiguousarray(COS2 * scale).astype(BF),
            np.ascontiguousarray(SINS2 * scale).astype(BF),
            np.ascontiguousarray(COS2).astype(BF),
            np.ascontiguousarray(SINS2).astype(BF))


def _prep_weights(patch_W, t_emb, Wuv, Wout, gnorm, fnorm_w, unpatch_W, nl=NL):
    Wg = Wuv[:nl] * gnorm[:nl, :, None]                  # fold gnorm
    u = Wg[:, :, :E]
    v = Wg[:, :, E:2 * E]
    q = Wg[:, :, 2 * E:2 * E + KD]
    k = Wg[:, :, 2 * E + KD:]
    wuvp = np.concatenate([u, q, k, v], axis=2)          # (nl, 768, 3328)
    w8 = np.clip(SW * wuvp, -240.0, 240.0).astype(F8)
    wuv_h = np.ascontiguousarray(
        w8.reshape(nl, HC, 128, WUV_W).transpose(0, 2, 1, 3))
    wout_h = np.ascontiguousarray(
        Wout[:nl].reshape(nl, EC, 128, H).transpose(0, 2, 1, 3)).astype(BF)
    pw_pad = np.zeros((256, H), np.float32)
    pw_pad[:PD] = patch_W
    pw_h = np.ascontiguousarray(
        pw_pad.reshape(2, 128, H).transpose(1, 0, 2)).astype(BF)
    upw = fnorm_w[:, None] * unpatch_W                   # fold fnorm
    upw_h = np.ascontiguousarray(
        upw.reshape(HC, 128, PD).transpose(1, 0, 2)).astype(BF)
    return wuv_h, wout_h, pw_h, upw_h


def _patchify(xc):
    """(2,3,128,128) -> (512, 192) token-major patches."""
    g = IMG // P
    xp = xc.reshape(2, 3, g, P, g, P).transpose(0, 2, 4, 3, 5, 1)
    return np.ascontiguousarray(xp.reshape(2 * L, PD))


def _unpatchify(oT):
    """(192, 512) -> (2, 3, 128, 128)."""
    g = IMG // P
    out = np.empty((2, 3, IMG, IMG), np.float32)
    for i in range(2):
        h = oT[:, i * L:(i + 1) * L].T                   # (256, 192)
        out[i] = (h.reshape(g, g, P, P, 3)
                  .transpose(4, 0, 2, 1, 3).reshape(3, IMG, IMG))
    return out


def make_in_maps(x, t_idx, patch_W, t_emb, Wuv, Wout, gnorm, fnorm_w,
                 unpatch_W, nl=NL):
    x = np.asarray(x, np.float32)
    t_idx = np.asarray(t_idx).astype(np.int64)
    patch_W = np.asarray(patch_W, np.float32)
    t_emb = np.asarray(t_emb, np.float32)
    Wuv = np.asarray(Wuv, np.float32)
    Wout = np.asarray(Wout, np.float32)
    gnorm = np.asarray(gnorm, np.float32)
    fnorm_w = np.asarray(fnorm_w, np.float32)
    unpatch_W = np.asarray(unpatch_W, np.float32)

    wuv_h, wout_h, pw_h, upw_h = _prep_weights(
        patch_W, t_emb, Wuv, Wout, gnorm, fnorm_w, unpatch_W, nl)
    cqt, sqt, ckt, skt = _rope_tables()
    sperm = np.ascontiguousarray(np.roll(np.eye(128, dtype=np.float32),
                                         64, axis=0)).astype(BF)

    in_maps = []
    for c in range(NCORES):
        xc = x[2 * c:2 * c + 2]
        xp = _patchify(xc)                               # (512, 192)
        xpad = np.zeros((TOK, 256), np.float32)
        xpad[:, :PD] = xp
        xpt = np.ascontiguousarray(
            xpad.T.reshape(2, 128, TOK).transpose(1, 0, 2)).astype(BF)
        te = t_emb[t_idx[2 * c:2 * c + 2, 0]]            # (2, 768)
        tembT = np.ascontiguousarray(
            te.T.reshape(HC, 128, 2).transpose(1, 0, 2)).astype(np.float32)
        in_maps.append({
            "xpt": xpt, "temb": tembT, "pw": pw_h, "wuv": wuv_h,
            "wout": wout_h, "upw": upw_h, "cq": cqt, "sq": sqt,
            "ck": ckt, "sk": skt, "sperm": sperm,
        })
    return in_maps


def kernel(**inputs):
    from concourse.bass_utils import run_bass_kernel_spmd

    nc = _get_nc()
    in_maps = make_in_maps(**inputs)
    res = run_bass_kernel_spmd(nc, in_maps, core_ids=list(range(NCORES)))
    out = np.empty((B, 3, IMG, IMG), np.float32)
    for c in range(NCORES):
        out[2 * c:2 * c + 2] = _unpatchify(res.results[c]["outt"])
    return out


# revision 6
# speedup vs baseline: 1.0944x; 1.0944x over previous
"""GAU denoising transformer forward pass on 8 Trainium2 NeuronCores.

Data-parallel over batch (B=16 -> 2 images per core); identical NEFF per
core. Residual stream hT kept fp32 in SBUF, transposed (H on partitions x
512 tokens). Big GEMMs run fp8-e4m3, mostly with DoubleRow perf mode (two
K=128 chunks contracted per instruction):

  - wuv8 = e4m3(64 * Wuv * gnorm) streamed from HBM (half the bytes of
    bf16). All of u/v/q/k read h8raw = e4m3(h) (raw residual, so no GEMM
    ever waits on the rms stats); the 1/rms and the 1/64 fp8 descale are
    folded into the Newton-rsqrt output (rt/rb) and applied at psum
    evacuation, exactly like the bf16 baseline did.
  - attention is computed TRANSPOSED (scores [m,l] via k-stationary
    matmuls): softmax sums reduce over partitions with a ones-matmul, exp
    stays unnormalized in fp8, attn@v directly produces oT (feature on
    partitions) via DoubleRow, no PE transposes. The 1/sum(exp) row goes
    through the PE scatter -> DVE reciprocal -> PE gather shuffle and is
    folded in after the out-projection.
  - v's GEMM is activation-stationary with the stationary h8 pair reused
    across 3 moving blocks (amortizes DoubleRow's 256-column LDWEIGHTS).
  - out-projection stays bf16 (fp8 there costs too much accuracy).

ACT uses only Silu and Exp (plus copies): the per-layer table-set
sequence is [silu (v,u)] -> [exp (scores)], i.e. exactly 2 table loads
per layer, and the order is forced by real data dependencies. Squares
and fp8 casts of h run on gpsimd; rms rsqrt is a DVE Newton iteration on
a (128,4) shuffle of the sumsq row (no Ln/Rsqrt tables).
"""

import sys

for _p in ("/opt/trn_rl_repo",):
    if _p not in sys.path:
        sys.path.append(_p)

import numpy as np
import ml_dtypes

BF = ml_dtypes.bfloat16
F8 = ml_dtypes.float8_e4m3

IMG = 128
P = 8
H = 768
E = 1536
KD = 128          # key size
L = 256           # patches per image
PD = 192          # patch dim
NL = 24
B = 16
NCORES = 8
TOK = 512         # tokens per core (2 images x 256)
HC = H // 128     # 6 h-chunks
EC = E // 128     # 12 e-chunks
SW = 64.0         # fp8 weight scale
WUV_W = 2 * E + 2 * KD    # permuted wuv width: u | q | k | v
QO = E                    # q col offset
KO = E + KD
V0 = E + 2 * KD


def _build(nl=NL, repeat=1):
    """Build + compile the Bass module. Returns nc."""
    import concourse.tile as tile
    from concourse import bacc, mybir
    from concourse.masks import make_identity

    F32 = mybir.dt.float32
    BF16 = mybir.dt.bfloat16
    FP8 = mybir.dt.float8e4
    AF = mybir.ActivationFunctionType
    DR = mybir.MatmulPerfMode.DoubleRow

    nc = bacc.Bacc("TRN2", target_bir_lowering=False, debug=False,
                   num_devices=NCORES)

    d_xpt = nc.dram_tensor("xpt", [128, 2, TOK], BF16, kind="ExternalInput")
    d_temb = nc.dram_tensor("temb", [128, HC, 2], F32, kind="ExternalInput")
    d_pw = nc.dram_tensor("pw", [128, 2, H], BF16, kind="ExternalInput")
    d_wuv = nc.dram_tensor("wuv", [nl, 128, HC, WUV_W], FP8,
                           kind="ExternalInput")
    d_wout = nc.dram_tensor("wout", [nl, 128, EC, H], BF16,
                            kind="ExternalInput")
    d_upw = nc.dram_tensor("upw", [128, HC, PD], BF16, kind="ExternalInput")
    d_sperm = nc.dram_tensor("sperm", [128, 128], BF16, kind="ExternalInput")
    d_cq = nc.dram_tensor("cq", [128, TOK], BF16, kind="ExternalInput")
    d_sq = nc.dram_tensor("sq", [128, TOK], BF16, kind="ExternalInput")
    d_ck = nc.dram_tensor("ck", [128, TOK], BF16, kind="ExternalInput")
    d_sk = nc.dram_tensor("sk", [128, TOK], BF16, kind="ExternalInput")
    d_out = nc.dram_tensor("outt", [PD, TOK], F32, kind="ExternalOutput")

    from contextlib import ExitStack

    with tile.TileContext(nc) as tc, ExitStack() as ctx:
        pers = ctx.enter_context(tc.tile_pool(name="pers", bufs=1))
        wuvp = ctx.enter_context(tc.tile_pool(name="wuvp", bufs=2))
        woutp = ctx.enter_context(tc.tile_pool(name="woutp", bufs=2))
        rtmp = ctx.enter_context(tc.tile_pool(name="rtmp", bufs=2))
        utmp = ctx.enter_context(tc.tile_pool(name="utmp", bufs=3))
        rmsp = ctx.enter_context(tc.tile_pool(name="rmsp", bufs=2))
        rbp = ctx.enter_context(tc.tile_pool(name="rbp", bufs=2))

        psum = ctx.enter_context(tc.tile_pool(name="psum", bufs=1, space="PSUM"))

        # ---- persistent state + constants ----
        hT = [pers.tile([128, TOK], F32, name=f"hT{j}", tag=f"hT{j}")
              for j in range(HC)]
        h8raw = pers.tile([128, HC, TOK], FP8)
        hsq8 = pers.tile([128, HC, TOK], FP8)
        uT = [pers.tile([128, TOK], BF16, name=f"uT{e}", tag=f"uT{e}")
              for e in range(EC)]
        ogT = [pers.tile([128, TOK], BF16, name=f"ogT{e}", tag=f"ogT{e}")
               for e in range(EC)]
        vg8 = pers.tile([128, 4, E], FP8)
        exp8 = [pers.tile([128, 2, 256], FP8, name=f"exp8_{i}", tag=f"exp8_{i}")
                for i in range(2)]
        qp = pers.tile([128, TOK], BF16)
        kp = pers.tile([128, TOK], BF16)
        qsb = pers.tile([128, TOK], BF16)
        ksb = pers.tile([128, TOK], BF16)
        cq = pers.tile([128, TOK], BF16)
        sq = pers.tile([128, TOK], BF16)
        ck = pers.tile([128, TOK], BF16)
        sk = pers.tile([128, TOK], BF16)
        rb = pers.tile([128, TOK], F32)     # (1/rms)/64 broadcast
        rcb = pers.tile([128, TOK], F32)    # 1/sum(exp) broadcast
        temb = pers.tile([128, HC, 2], F32)
        xpt = pers.tile([128, 2, TOK], BF16)
        pw = pers.tile([128, 2, H], BF16)
        upw = pers.tile([128, HC, PD], BF16)
        sperm = pers.tile([128, 128], BF16)
        ones8 = pers.tile([128, 2, 16], FP8)
        twos8 = pers.tile([128, 2, 16], FP8)
        ones1 = pers.tile([1, 1], F32)
        identf = pers.tile([128, 128], F32)
        bln4 = pers.tile([128, 1], F32)     # -ln(4)

        nc.sync.dma_start(cq, d_cq.ap())
        nc.sync.dma_start(sq, d_sq.ap())
        nc.sync.dma_start(ck, d_ck.ap())
        nc.sync.dma_start(sk, d_sk.ap())
        nc.sync.dma_start(temb, d_temb.ap())
        nc.sync.dma_start(xpt, d_xpt.ap())
        nc.sync.dma_start(pw, d_pw.ap())
        nc.sync.dma_start(upw, d_upw.ap())
        nc.sync.dma_start(sperm, d_sperm.ap())
        nc.vector.memset(ones8, 1.0)
        nc.vector.memset(twos8, 2.0)
        nc.vector.memset(ones1, 1.0)
        nc.vector.memset(bln4, float(-np.log(4.0)))
        make_identity(nc, identf)

        state = {}

        def tail_chunk(hp):
            """After hT[hp] is final: fp8 cast + square on gpsimd, sumsq
            pair-matmuls (DoubleRow, x2 via twos8) into the `prow` row."""
            nc.gpsimd.tensor_copy(h8raw[:, hp, :], hT[hp])
            nc.gpsimd.tensor_mul(hsq8[:, hp, :], h8raw[:, hp, :],
                                 h8raw[:, hp, :])
            if hp % 2 == 1:
                t = hp // 2
                nc.tensor.matmul(state["ss"], twos8[:, :, 0:1],
                                 hsq8[:, hp - 1:hp + 1, :],
                                 start=(t == 0), stop=(t == 2),
                                 perf_mode=DR)

        def new_ss(name):
            state["ss"] = psum.tile([1, TOK], F32, tag="prow",
                                    name=f"ss_{name}")

        def rms_stats(name):
            """(1/rms)/64 from the sumsq row: (1,TOK)->(128,4) PE scatter,
            DVE Newton rsqrt, PE gather, gpsimd broadcast. Returns rt
            (128,4) with column t = (1/rms)/64 of token block t (the /64
            descales the fp8 weight scale)."""
            ssb = rmsp.tile([1, TOK], F32, tag="ssb", name=f"ssb_{name}")
            nc.scalar.copy(ssb, state["ss"])
            sst = psum.tile([128, 4], F32, tag="pst", name=f"sst_{name}")
            for t in range(4):
                nc.tensor.matmul(sst[:, t:t + 1], ssb[:, t * 128:(t + 1) * 128],
                                 ones1, start=True, stop=True)
            # m = 16*mean(h^2); clamp to the seed's convergence window;
            # rt = rsqrt(m)*4/64 = (1/rms)/64.
            m = rmsp.tile([128, 4], F32, tag="m", name=f"m_{name}")
            nc.vector.tensor_scalar(m, sst, 8.0 / H, None,
                                    mybir.AluOpType.mult)
            nc.vector.tensor_scalar(m, m, 0.15, 6.0,
                                    mybir.AluOpType.max,
                                    mybir.AluOpType.min)
            rt = rbp.tile([128, 4], F32, tag="rt", name=f"rt_{name}")
            t1 = rmsp.tile([128, 4], F32, tag="t1", name=f"t1_{name}")
            nc.vector.reciprocal(rt, m)
            nc.vector.tensor_scalar(rt, rt, 0.5, 0.5,
                                    mybir.AluOpType.mult,
                                    mybir.AluOpType.add)
            for _ in range(4):   # newton: y *= 1.5 - 0.5*m*y^2
                nc.vector.tensor_mul(t1, rt, rt)
                nc.vector.tensor_mul(t1, t1, m)
                nc.vector.tensor_scalar(t1, t1, -0.5, 1.5,
                                        mybir.AluOpType.mult,
                                        mybir.AluOpType.add)
                nc.vector.tensor_mul(rt, rt, t1)
            nc.vector.tensor_scalar_mul(rt, rt, 4.0 / SW)
            row = psum.tile([1, TOK], F32, tag="prow", name=f"row_{name}")
            for t in range(4):
                nc.tensor.matmul(row[:, t * 128:(t + 1) * 128], rt[:, t:t + 1],
                                 identf, start=True, stop=True)
            yrow = rmsp.tile([1, TOK], F32, tag="yrow", name=f"yrow_{name}")
            nc.scalar.copy(yrow, row)
            nc.gpsimd.partition_broadcast(rb, yrow)
            return rt

        # ---- patchify: hT = patch_W.T @ xp.T + temb ----
        for j in range(HC):
            ps = psum.tile([128, TOK], F32, tag=("pa", "pb")[j % 2])
            for c in range(2):
                nc.tensor.matmul(ps, pw[:, c, j * 128:(j + 1) * 128],
                                 xpt[:, c, :], start=(c == 0), stop=(c == 1))
            for i in range(2):
                nc.vector.tensor_scalar_add(
                    hT[j][:, i * 256:(i + 1) * 256],
                    ps[:, i * 256:(i + 1) * 256],
                    temb[:, j, i:i + 1])
        new_ss("init")
        for j in range(HC):
            tail_chunk(j)

        ROT = ["pa", "pb", "pqs", "pks"]

        for lrep in range(nl * repeat):
            li = lrep % nl
            wuv = wuvp.tile([128, HC, WUV_W], FP8, tag="wuv")
            nc.sync.dma_start(wuv, d_wuv.ap()[li])
            wout = woutp.tile([128, EC, H], BF16, tag="wout")
            nc.sync.dma_start(wout, d_wout.ap()[li])

            # rms stats for this layer's activations (off the GEMM critical
            # path: all GEMMs read raw h8; rt/rb land at psum evacuation)
            rt = rms_stats(f"l{lrep}")

            # ---- q/k col-tiles (DoubleRow on raw h8) + swap via perm ----
            q_ps = psum.tile([128, TOK], F32, tag="pq")
            k_ps = psum.tile([128, TOK], F32, tag="pk")
            for ps, c0 in ((q_ps, QO), (k_ps, KO)):
                for t in range(3):
                    nc.tensor.matmul(ps, wuv[:, 2 * t:2 * t + 2, c0:c0 + 128],
                                     h8raw[:, 2 * t:2 * t + 2, :],
                                     start=(t == 0), stop=(t == 2),
                                     perf_mode=DR)
            nc.scalar.copy(qsb, q_ps)
            nc.scalar.copy(ksb, k_ps)
            qs_ps = psum.tile([128, TOK], F32, tag="pqs")
            nc.tensor.matmul(qs_ps, sperm, qsb, start=True, stop=True)
            ks_ps = psum.tile([128, TOK], F32, tag="pks")
            nc.tensor.matmul(ks_ps, sperm, ksb, start=True, stop=True)

            # ---- rope (x (1/rms)/64): q' = (Q*cos + Qswap*sins) * rb ----
            m1 = rtmp.tile([128, TOK], F32, tag="m1")
            m2 = rtmp.tile([128, TOK], F32, tag="m2")
            nc.vector.tensor_mul(m1, q_ps, cq)
            nc.vector.tensor_mul(m2, qs_ps, sq)
            nc.vector.tensor_add(m1, m1, m2)
            nc.vector.tensor_mul(qp, m1, rb)
            m3 = rtmp.tile([128, TOK], F32, tag="m1")
            m4 = rtmp.tile([128, TOK], F32, tag="m2")
            nc.vector.tensor_mul(m3, k_ps, ck)
            nc.vector.tensor_mul(m4, ks_ps, sk)
            nc.vector.tensor_add(m3, m3, m4)
            nc.vector.tensor_mul(kp, m3, rb)

            # ---- v natural (h8 stationary reused over 3 moving blocks,
            #      DoubleRow); silu((1/rms)/64 * ps) with per-partition rt ----
            for tk in range(4):
                vps = [psum.tile([128, 512], F32, tag=ROT[(tk * 3 + ns) % 4],
                                 name=f"v{lrep}_{tk}_{ns}")
                       for ns in range(3)]
                for t in range(3):
                    for ns in range(3):
                        nc.tensor.matmul(
                            vps[ns],
                            h8raw[:, 2 * t:2 * t + 2, tk * 128:(tk + 1) * 128],
                            wuv[:, 2 * t:2 * t + 2,
                                V0 + ns * 512:V0 + (ns + 1) * 512],
                            start=(t == 0), stop=(t == 2), perf_mode=DR)
                for ns in range(3):
                    nc.scalar.activation(vg8[:, tk, ns * 512:(ns + 1) * 512],
                                         vps[ns], AF.Silu,
                                         scale=rt[:, tk:tk + 1])

            # ---- u col-tiles (weight-stationary, DoubleRow) ----
            for ct in range(EC):
                ps = psum.tile([128, TOK], F32, tag=ROT[ct % 4])
                for t in range(3):
                    nc.tensor.matmul(ps,
                                     wuv[:, 2 * t:2 * t + 2,
                                         ct * 128:(ct + 1) * 128],
                                     h8raw[:, 2 * t:2 * t + 2, :],
                                     start=(t == 0), stop=(t == 2),
                                     perf_mode=DR)
                ut = utmp.tile([128, TOK], BF16, tag="ut")
                nc.vector.tensor_mul(ut, ps, rb)
                nc.scalar.activation(uT[ct], ut, AF.Silu)

            # ---- transposed scores + exp (fp8, unnormalized) ----
            for i in range(2):
                scp = psum.tile([128, 2, 256], F32, tag=("pq", "pk")[i])
                for mc in range(2):
                    nc.tensor.matmul(scp[:, mc, :],
                                     kp[:, i * 256 + mc * 128:
                                        i * 256 + (mc + 1) * 128],
                                     qp[:, i * 256:(i + 1) * 256],
                                     start=True, stop=True)
                nc.scalar.activation(exp8[i], scp, AF.Exp, scale=1.0,
                                     bias=bln4[:, 0:1])

            # ---- softmax sums over m -> 1/sum row -> rcb ----
            srow = psum.tile([1, TOK], F32, tag="prow", name=f"sr_{lrep}")
            for i in range(2):
                nc.tensor.matmul(srow[:, i * 256:(i + 1) * 256],
                                 ones8[:, :, 0:1], exp8[i],
                                 start=True, stop=True, perf_mode=DR)
            srb = rmsp.tile([1, TOK], F32, tag="srb", name=f"srb_{lrep}")
            nc.scalar.copy(srb, srow)
            sct = psum.tile([128, 4], F32, tag="pst", name=f"sct_{lrep}")
            for t in range(4):
                nc.tensor.matmul(sct[:, t:t + 1], srb[:, t * 128:(t + 1) * 128],
                                 ones1, start=True, stop=True)
            rec = rbp.tile([128, 4], F32, tag="rec", name=f"rec_{lrep}")
            nc.vector.reciprocal(rec, sct)
            rrow = psum.tile([1, TOK], F32, tag="prow", name=f"rr_{lrep}")
            for t in range(4):
                nc.tensor.matmul(rrow[:, t * 128:(t + 1) * 128], rec[:, t:t + 1],
                                 identf, start=True, stop=True)
            rrb = rmsp.tile([1, TOK], F32, tag="rrb", name=f"rrb_{lrep}")
            nc.scalar.copy(rrb, rrow)
            nc.gpsimd.partition_broadcast(rcb, rrb)

            # ---- oT = (exp @ v).T via v-stationary DoubleRow; gate ----
            for e in range(EC):
                ops = psum.tile([128, TOK], F32, tag=ROT[e % 4])
                for i in range(2):
                    nc.tensor.matmul(ops[:, i * 256:(i + 1) * 256],
                                     vg8[:, 2 * i:2 * i + 2,
                                         e * 128:(e + 1) * 128],
                                     exp8[i], start=True, stop=True,
                                     perf_mode=DR)
                nc.vector.tensor_mul(ogT[e], uT[e], ops)

            # ---- out-projection (bf16) + attn-normalize + residual ----
            new_ss(f"l{lrep}")
            for hp in range(HC):
                dps = psum.tile([128, TOK], F32, tag=ROT[hp % 4])
                for e in range(EC):
                    nc.tensor.matmul(dps, wout[:, e, hp * 128:(hp + 1) * 128],
                                     ogT[e], start=(e == 0), stop=(e == EC - 1))
                tmp = rtmp.tile([128, TOK], F32, tag="rtm")
                nc.vector.tensor_mul(tmp, dps, rcb)
                nc.vector.tensor_add(hT[hp], hT[hp], tmp)
                tail_chunk(hp)

        # ---- final norm + unpatch (fnorm_w folded into upw on host) ----
        rt = rms_stats("fin")
        hfin = uT   # reuse dead uT tiles as bf16 normalized h
        for j in range(HC):
            # rb is (1/rms)/64; upw is scaled x64 on host to compensate
            nc.vector.tensor_mul(hfin[j], hT[j], rb)
        for mchunk, msz in ((0, 128), (1, 64)):
            ps = psum.tile([128, TOK], F32, tag=("pa", "pb")[mchunk])
            for j in range(HC):
                nc.tensor.matmul(ps[:msz, :],
                                 upw[:, j, mchunk * 128:mchunk * 128 + msz],
                                 hfin[j], start=(j == 0), stop=(j == HC - 1))
            osb = rtmp.tile([128, TOK], F32, tag="m1")
            nc.vector.tensor_copy(osb[:msz, :], ps[:msz, :])
            nc.sync.dma_start(d_out.ap()[mchunk * 128:mchunk * 128 + msz, :],
                              osb[:msz, :])

    nc.compile()
    return nc


_BUILD_CACHE = {}


def _get_nc(nl=NL, repeat=1):
    key = (nl, repeat)
    if key not in _BUILD_CACHE:
        _BUILD_CACHE[key] = _build(nl, repeat)
    return _BUILD_CACHE[key]


def _rope_tables():
    pos = np.arange(L)

    def sinemb(p, dim=64, base=1000.0):
        half = dim // 2
        freqs = np.exp(np.arange(half, dtype=np.float32)
                       * np.float32(-np.log(base) / (half - 1)))
        ang = p[:, None].astype(np.float32) * freqs[None, :]
        return np.concatenate([np.sin(ang), np.cos(ang)], axis=-1)

    w = IMG // P
    pe = np.concatenate([sinemb(pos // w), sinemb(pos % w)],
                        axis=-1).astype(np.float32)      # (256, 128)
    sinv = pe[:, :64].T                                  # (64, 256)
    cosv = pe[:, 64:].T
    COS = np.concatenate([cosv, cosv], axis=0)           # (128, 256)
    SINS = np.concatenate([-sinv, sinv], axis=0)
    COS2 = np.tile(COS, (1, 2))                          # (128, 512)
    SINS2 = np.tile(SINS, (1, 2))
    scale = np.float32(KD ** -0.5)
    return (np.ascontiguousarray(COS2 * scale).astype(BF),
            np.ascontiguousarray(SINS2 * scale).astype(BF),
            np.ascontiguousarray(COS2).astype(BF),
            np.ascontiguousarray(SINS2).astype(BF))


def _prep_weights(patch_W, t_emb, Wuv, Wout, gnorm, fnorm_w, unpatch_W, nl=NL):
    Wg = Wuv[:nl] * gnorm[:nl, :, None]                  # fold gnorm
    u = Wg[:, :, :E]
    v = Wg[:, :, E:2 * E]
    q = Wg[:, :, 2 * E:2 * E + KD]
    k = Wg[:, :, 2 * E + KD:]
    wuvp = np.concatenate([u, q, k, v], axis=2)          # (nl, 768, 3328)
    w8 = np.clip(SW * wuvp, -240.0, 240.0).astype(F8)
    wuv_h = np.ascontiguousarray(
        w8.reshape(nl, HC, 128, WUV_W).transpose(0, 2, 1, 3))
    wout_h = np.ascontiguousarray(
        Wout[:nl].reshape(nl, EC, 128, H).transpose(0, 2, 1, 3)).astype(BF)
    pw_pad = np.zeros((256, H), np.float32)
    pw_pad[:PD] = patch_W
    pw_h = np.ascontiguousarray(
        pw_pad.reshape(2, 128, H).transpose(1, 0, 2)).astype(BF)
    # final rb carries a 1/64 descale (shared Newton path); fold x64 here
    upw = SW * fnorm_w[:, None] * unpatch_W
    upw_h = np.ascontiguousarray(
        upw.reshape(HC, 128, PD).transpose(1, 0, 2)).astype(BF)
    return wuv_h, wout_h, pw_h, upw_h


def _patchify(xc):
    """(2,3,128,128) -> (512, 192) token-major patches."""
    g = IMG // P
    xp = xc.reshape(2, 3, g, P, g, P).transpose(0, 2, 4, 3, 5, 1)
    return np.ascontiguousarray(xp.reshape(2 * L, PD))


def _unpatchify(oT):
    """(192, 512) -> (2, 3, 128, 128)."""
    g = IMG // P
    out = np.empty((2, 3, IMG, IMG), np.float32)
    for i in range(2):
        h = oT[:, i * L:(i + 1) * L].T                   # (256, 192)
        out[i] = (h.reshape(g, g, P, P, 3)
                  .transpose(4, 0, 2, 1, 3).reshape(3, IMG, IMG))
    return out


def make_in_maps(x, t_idx, patch_W, t_emb, Wuv, Wout, gnorm, fnorm_w,
                 unpatch_W, nl=NL):
    x = np.asarray(x, np.float32)
    t_idx = np.asarray(t_idx).astype(np.int64)
    patch_W = np.asarray(patch_W, np.float32)
    t_emb = np.asarray(t_emb, np.float32)
    Wuv = np.asarray(Wuv, np.float32)
    Wout = np.asarray(Wout, np.float32)
    gnorm = np.asarray(gnorm, np.float32)
    fnorm_w = np.asarray(fnorm_w, np.float32)
    unpatch_W = np.asarray(unpatch_W, np.float32)

    wuv_h, wout_h, pw_h, upw_h = _prep_weights(
        patch_W, t_emb, Wuv, Wout, gnorm, fnorm_w, unpatch_W, nl)
    cqt, sqt, ckt, skt = _rope_tables()
    sperm = np.ascontiguousarray(np.roll(np.eye(128, dtype=np.float32),
                                         64, axis=0)).astype(BF)

    in_maps = []
    for c in range(NCORES):
        xc = x[2 * c:2 * c + 2]
        xp = _patchify(xc)                               # (512, 192)
        xpad = np.zeros((TOK, 256), np.float32)
        xpad[:, :PD] = xp
        xpt = np.ascontiguousarray(
            xpad.T.reshape(2, 128, TOK).transpose(1, 0, 2)).astype(BF)
        te = t_emb[t_idx[2 * c:2 * c + 2, 0]]            # (2, 768)
        tembT = np.ascontiguousarray(
            te.T.reshape(HC, 128, 2).transpose(1, 0, 2)).astype(np.float32)
        in_maps.append({
            "xpt": xpt, "temb": tembT, "pw": pw_h, "wuv": wuv_h,
            "wout": wout_h, "upw": upw_h, "cq": cqt, "sq": sqt,
            "ck": ckt, "sk": skt, "sperm": sperm,
        })
    return in_maps


def kernel(**inputs):
    from concourse.bass_utils import run_bass_kernel_spmd

    nc = _get_nc()
    in_maps = make_in_maps(**inputs)
    res = run_bass_kernel_spmd(nc, in_maps, core_ids=list(range(NCORES)))
    out = np.empty((B, 3, IMG, IMG), np.float32)
    for c in range(NCORES):
        out[2 * c:2 * c + 2] = _unpatchify(res.results[c]["outt"])
    return out


# revision 10
# speedup vs baseline: 1.3857x; 1.2662x over previous
"""GAU denoising transformer forward pass on 8 Trainium2 NeuronCores.

Data-parallel over batch (B=16 -> 2 images per core); identical NEFF per
core. Residual stream hT kept fp32 in SBUF, transposed (H on partitions x
512 tokens). Big GEMMs run fp8-e4m3, mostly with DoubleRow perf mode (two
K=128 chunks contracted per instruction):

  - wuv8 = e4m3(64 * Wuv * gnorm) streamed from HBM (half the bytes of
    bf16). All of u/v/q/k read h8raw = e4m3(h) (raw residual, so no GEMM
    ever waits on the rms stats); the 1/rms and the 1/64 fp8 descale are
    folded into the Newton-rsqrt output (rt/rb) and applied at psum
    evacuation, exactly like the bf16 baseline did.
  - attention is computed TRANSPOSED (scores [m,l] via k-stationary
    matmuls): softmax sums reduce over partitions with a ones-matmul, exp
    stays unnormalized in fp8, attn@v directly produces oT (feature on
    partitions) via DoubleRow, no PE transposes. The 1/sum(exp) row goes
    through the PE scatter -> DVE reciprocal -> PE gather shuffle and is
    folded in after the out-projection.
  - v's GEMM is activation-stationary with the stationary h8 pair reused
    across 3 moving blocks (amortizes DoubleRow's 256-column LDWEIGHTS).
  - out-projection stays bf16 (fp8 there costs too much accuracy).

ACT uses only Silu and Exp (plus copies): the per-layer table-set
sequence is [silu (v,u)] -> [exp (scores)], i.e. exactly 2 table loads
per layer, and the order is forced by real data dependencies. Squares
and fp8 casts of h run on gpsimd; rms rsqrt is a DVE Newton iteration on
a (128,4) shuffle of the sumsq row (no Ln/Rsqrt tables).
"""

import sys

for _p in ("/opt/trn_rl_repo",):
    if _p not in sys.path:
        sys.path.append(_p)

import numpy as np
import ml_dtypes

BF = ml_dtypes.bfloat16
F8 = ml_dtypes.float8_e4m3

IMG = 128
P = 8
H = 768
E = 1536
KD = 128          # key size
L = 256           # patches per image
PD = 192          # patch dim
NL = 24
B = 16
NCORES = 8
TOK = 512         # tokens per core (2 images x 256)
HC = H // 128     # 6 h-chunks
EC = E // 128     # 12 e-chunks
SW = 64.0         # fp8 weight scale
WUV_W = 2 * E + 2 * KD    # permuted wuv width: u | q | k | v
QO = E                    # q col offset
KO = E + KD
V0 = E + 2 * KD


def _build(nl=NL, repeat=1):
    """Build + compile the Bass module. Returns nc."""
    import concourse.tile as tile
    from concourse import bacc, mybir
    from concourse.masks import make_identity

    F32 = mybir.dt.float32
    BF16 = mybir.dt.bfloat16
    FP8 = mybir.dt.float8e4
    AF = mybir.ActivationFunctionType
    DR = mybir.MatmulPerfMode.DoubleRow

    nc = bacc.Bacc("TRN2", target_bir_lowering=False, debug=False,
                   num_devices=NCORES)

    d_xpt = nc.dram_tensor("xpt", [128, 2, TOK], BF16, kind="ExternalInput")
    d_temb = nc.dram_tensor("temb", [128, HC, 2], F32, kind="ExternalInput")
    d_pw = nc.dram_tensor("pw", [128, 2, H], BF16, kind="ExternalInput")
    d_wuv = nc.dram_tensor("wuv", [nl, 128, HC, WUV_W], FP8,
                           kind="ExternalInput")
    d_wout = nc.dram_tensor("wout", [nl, 128, EC, H], BF16,
                            kind="ExternalInput")
    d_upw = nc.dram_tensor("upw", [128, HC, PD], BF16, kind="ExternalInput")
    d_sperm = nc.dram_tensor("sperm", [128, 128], BF16, kind="ExternalInput")
    d_cq = nc.dram_tensor("cq", [128, TOK], BF16, kind="ExternalInput")
    d_sq = nc.dram_tensor("sq", [128, TOK], BF16, kind="ExternalInput")
    d_ck = nc.dram_tensor("ck", [128, TOK], BF16, kind="ExternalInput")
    d_sk = nc.dram_tensor("sk", [128, TOK], BF16, kind="ExternalInput")
    d_out = nc.dram_tensor("outt", [PD, TOK], F32, kind="ExternalOutput")

    from contextlib import ExitStack

    with tile.TileContext(nc) as tc, ExitStack() as ctx:
        pers = ctx.enter_context(tc.tile_pool(name="pers", bufs=1))
        wuvp = ctx.enter_context(tc.tile_pool(name="wuvp", bufs=2))
        woutp = ctx.enter_context(tc.tile_pool(name="woutp", bufs=2))
        rtmp = ctx.enter_context(tc.tile_pool(name="rtmp", bufs=2))
        utmp = ctx.enter_context(tc.tile_pool(name="utmp", bufs=3))
        rmsp = ctx.enter_context(tc.tile_pool(name="rmsp", bufs=2))
        rbp = ctx.enter_context(tc.tile_pool(name="rbp", bufs=2))

        psum = ctx.enter_context(tc.tile_pool(name="psum", bufs=1, space="PSUM"))

        # ---- persistent state + constants ----
        hT = [pers.tile([128, TOK], F32, name=f"hT{j}", tag=f"hT{j}")
              for j in range(HC)]
        h8raw = pers.tile([128, HC, TOK], FP8)
        hsq8 = pers.tile([128, HC, TOK], FP8)
        uT = [pers.tile([128, TOK], BF16, name=f"uT{e}", tag=f"uT{e}")
              for e in range(EC)]
        ogT = [pers.tile([128, TOK], BF16, name=f"ogT{e}", tag=f"ogT{e}")
               for e in range(EC)]
        vg8 = pers.tile([128, 4, E], FP8)
        exp8 = [pers.tile([128, 2, 256], FP8, name=f"exp8_{i}", tag=f"exp8_{i}")
                for i in range(2)]
        qp = pers.tile([128, TOK], BF16)
        kp = pers.tile([128, TOK], BF16)
        qsb = pers.tile([128, TOK], BF16)
        ksb = pers.tile([128, TOK], BF16)
        cq = pers.tile([128, TOK], BF16)
        sq = pers.tile([128, TOK], BF16)
        ck = pers.tile([128, TOK], BF16)
        sk = pers.tile([128, TOK], BF16)
        rb = pers.tile([128, TOK], F32)     # (1/rms)/64 broadcast
        rcb = pers.tile([128, TOK], F32)    # 1/sum(exp) broadcast
        temb = pers.tile([128, HC, 2], F32)
        xpt = pers.tile([128, 2, TOK], BF16)
        pw = pers.tile([128, 2, H], BF16)
        upw = pers.tile([128, HC, PD], BF16)
        sperm = pers.tile([128, 128], BF16)
        ones8 = pers.tile([128, 2, 16], FP8)
        twos8 = pers.tile([128, 2, 16], FP8)
        ones1 = pers.tile([1, 1], F32)
        identf = pers.tile([128, 128], F32)
        bln4 = pers.tile([128, 1], F32)     # -ln(4)

        nc.sync.dma_start(cq, d_cq.ap())
        nc.sync.dma_start(sq, d_sq.ap())
        nc.sync.dma_start(ck, d_ck.ap())
        nc.sync.dma_start(sk, d_sk.ap())
        nc.sync.dma_start(temb, d_temb.ap())
        nc.sync.dma_start(xpt, d_xpt.ap())
        nc.sync.dma_start(pw, d_pw.ap())
        nc.sync.dma_start(upw, d_upw.ap())
        nc.sync.dma_start(sperm, d_sperm.ap())
        nc.vector.memset(ones8, 1.0)
        nc.vector.memset(twos8, 2.0)
        nc.vector.memset(ones1, 1.0)
        nc.vector.memset(bln4, float(-np.log(4.0)))
        make_identity(nc, identf)

        state = {}

        def tail_chunk(hp):
            """After hT[hp] is final: fp8 cast (DVE, fast) + square (ACT,
            present in every table set). The sumsq matmuls are deferred to
            the next layer's start so the PE never waits here."""
            nc.vector.tensor_copy(h8raw[:, hp, :], hT[hp])
            nc.scalar.activation(hsq8[:, hp, :], hT[hp], AF.Square)

        def sumsq(name):
            """Accumulate the sumsq row (x2 via twos8) from hsq8."""
            ss = psum.tile([1, TOK], F32, tag="prow", name=f"ss_{name}")
            for t in range(3):
                nc.tensor.matmul(ss, twos8[:, :, 0:1],
                                 hsq8[:, 2 * t:2 * t + 2, :],
                                 start=(t == 0), stop=(t == 2),
                                 perf_mode=DR)
            state["ss"] = ss

        def rms_stats(name):
            """(1/rms)/64 from the sumsq row: (1,TOK)->(128,4) PE scatter,
            DVE Newton rsqrt, PE gather, gpsimd broadcast. Returns rt
            (128,4) with column t = (1/rms)/64 of token block t (the /64
            descales the fp8 weight scale)."""
            ssb = rmsp.tile([1, TOK], F32, tag="ssb", name=f"ssb_{name}")
            nc.scalar.copy(ssb, state["ss"])
            sst = psum.tile([128, 4], F32, tag="pst", name=f"sst_{name}")
            for t in range(4):
                nc.tensor.matmul(sst[:, t:t + 1], ssb[:, t * 128:(t + 1) * 128],
                                 ones1, start=True, stop=True)
            # m = 16*mean(h^2); clamp to the seed's convergence window;
            # rt = rsqrt(m)*4/64 = (1/rms)/64.
            m = rmsp.tile([128, 4], F32, tag="m", name=f"m_{name}")
            nc.vector.tensor_scalar(m, sst, 8.0 / H, None,
                                    mybir.AluOpType.mult)
            nc.vector.tensor_scalar(m, m, 0.15, 6.0,
                                    mybir.AluOpType.max,
                                    mybir.AluOpType.min)
            rt = rbp.tile([128, 4], F32, tag="rt", name=f"rt_{name}")
            t1 = rmsp.tile([128, 4], F32, tag="t1", name=f"t1_{name}")
            nc.vector.reciprocal(rt, m)
            nc.vector.tensor_scalar(rt, rt, 0.5, 0.5,
                                    mybir.AluOpType.mult,
                                    mybir.AluOpType.add)
            for _ in range(4):   # newton: y *= 1.5 - 0.5*m*y^2
                nc.vector.tensor_mul(t1, rt, rt)
                nc.vector.tensor_mul(t1, t1, m)
                nc.vector.tensor_scalar(t1, t1, -0.5, 1.5,
                                        mybir.AluOpType.mult,
                                        mybir.AluOpType.add)
                nc.vector.tensor_mul(rt, rt, t1)
            nc.vector.tensor_scalar_mul(rt, rt, 4.0 / SW)
            row = psum.tile([1, TOK], F32, tag="prow", name=f"row_{name}")
            for t in range(4):
                nc.tensor.matmul(row[:, t * 128:(t + 1) * 128], rt[:, t:t + 1],
                                 identf, start=True, stop=True)
            yrow = rmsp.tile([1, TOK], F32, tag="yrow", name=f"yrow_{name}")
            nc.scalar.copy(yrow, row)
            nc.gpsimd.partition_broadcast(rb, yrow)
            return rt

        # ---- patchify: hT = patch_W.T @ xp.T + temb ----
        for j in range(HC):
            ps = psum.tile([128, TOK], F32, tag=("pa", "pb")[j % 2])
            for c in range(2):
                nc.tensor.matmul(ps, pw[:, c, j * 128:(j + 1) * 128],
                                 xpt[:, c, :], start=(c == 0), stop=(c == 1))
            for i in range(2):
                nc.vector.tensor_scalar_add(
                    hT[j][:, i * 256:(i + 1) * 256],
                    ps[:, i * 256:(i + 1) * 256],
                    temb[:, j, i:i + 1])
        for j in range(HC):
            tail_chunk(j)

        ROT = ["pa", "pb", "pqs", "pks"]

        for lrep in range(nl * repeat):
            li = lrep % nl
            wuv = wuvp.tile([128, HC, WUV_W], FP8, tag="wuv")
            nc.sync.dma_start(wuv, d_wuv.ap()[li])
            wout = woutp.tile([128, EC, H], BF16, tag="wout")
            nc.sync.dma_start(wout, d_wout.ap()[li])

            # ---- q/k col-tiles (DoubleRow on raw h8) + swap via perm ----
            q_ps = psum.tile([128, TOK], F32, tag="pq")
            k_ps = psum.tile([128, TOK], F32, tag="pk")
            for ps, c0 in ((q_ps, QO), (k_ps, KO)):
                for t in range(3):
                    nc.tensor.matmul(ps, wuv[:, 2 * t:2 * t + 2, c0:c0 + 128],
                                     h8raw[:, 2 * t:2 * t + 2, :],
                                     start=(t == 0), stop=(t == 2),
                                     perf_mode=DR)
            nc.scalar.copy(qsb, q_ps)
            nc.scalar.copy(ksb, k_ps)
            qs_ps = psum.tile([128, TOK], F32, tag="pqs")
            nc.tensor.matmul(qs_ps, sperm, qsb, start=True, stop=True)
            ks_ps = psum.tile([128, TOK], F32, tag="pks")
            nc.tensor.matmul(ks_ps, sperm, ksb, start=True, stop=True)

            # rms stats for this layer (off the GEMM critical path: all
            # GEMMs read raw h8; rt/rb land at psum evacuation)
            sumsq(f"l{lrep}")
            rt = rms_stats(f"l{lrep}")

            # ---- rope (x (1/rms)/64): q' = (Q*cos + Qswap*sins) * rb ----
            m1 = rtmp.tile([128, TOK], F32, tag="m1")
            m2 = rtmp.tile([128, TOK], F32, tag="m2")
            nc.vector.tensor_mul(m1, q_ps, cq)
            nc.vector.tensor_mul(m2, qs_ps, sq)
            nc.vector.tensor_add(m1, m1, m2)
            nc.vector.tensor_mul(qp, m1, rb)
            m3 = rtmp.tile([128, TOK], F32, tag="m1")
            m4 = rtmp.tile([128, TOK], F32, tag="m2")
            nc.vector.tensor_mul(m3, k_ps, ck)
            nc.vector.tensor_mul(m4, ks_ps, sk)
            nc.vector.tensor_add(m3, m3, m4)
            nc.vector.tensor_mul(kp, m3, rb)

            # ---- v natural (h8 stationary reused over 3 moving blocks,
            #      DoubleRow); silu((1/rms)/64 * ps) with per-partition rt ----
            for tk in range(4):
                vps = [psum.tile([128, 512], F32, tag=ROT[(tk * 3 + ns) % 4],
                                 name=f"v{lrep}_{tk}_{ns}")
                       for ns in range(3)]
                for t in range(3):
                    for ns in range(3):
                        nc.tensor.matmul(
                            vps[ns],
                            h8raw[:, 2 * t:2 * t + 2, tk * 128:(tk + 1) * 128],
                            wuv[:, 2 * t:2 * t + 2,
                                V0 + ns * 512:V0 + (ns + 1) * 512],
                            start=(t == 0), stop=(t == 2), perf_mode=DR)
                for ns in range(3):
                    nc.scalar.activation(vg8[:, tk, ns * 512:(ns + 1) * 512],
                                         vps[ns], AF.Silu,
                                         scale=rt[:, tk:tk + 1])

            # ---- u col-tiles (weight-stationary, DoubleRow) ----
            for ct in range(EC):
                ps = psum.tile([128, TOK], F32, tag=ROT[ct % 4])
                for t in range(3):
                    nc.tensor.matmul(ps,
                                     wuv[:, 2 * t:2 * t + 2,
                                         ct * 128:(ct + 1) * 128],
                                     h8raw[:, 2 * t:2 * t + 2, :],
                                     start=(t == 0), stop=(t == 2),
                                     perf_mode=DR)
                ut = utmp.tile([128, TOK], BF16, tag="ut")
                nc.vector.tensor_mul(ut, ps, rb)
                nc.scalar.activation(uT[ct], ut, AF.Silu)

            # ---- transposed scores + exp (fp8, unnormalized) ----
            for i in range(2):
                scp = psum.tile([128, 2, 256], F32, tag=("pq", "pk")[i])
                for mc in range(2):
                    nc.tensor.matmul(scp[:, mc, :],
                                     kp[:, i * 256 + mc * 128:
                                        i * 256 + (mc + 1) * 128],
                                     qp[:, i * 256:(i + 1) * 256],
                                     start=True, stop=True)
                nc.scalar.activation(exp8[i], scp, AF.Exp, scale=1.0,
                                     bias=bln4[:, 0:1])

            # ---- softmax sums over m -> 1/sum row -> rcb ----
            srow = psum.tile([1, TOK], F32, tag="prow", name=f"sr_{lrep}")
            for i in range(2):
                nc.tensor.matmul(srow[:, i * 256:(i + 1) * 256],
                                 ones8[:, :, 0:1], exp8[i],
                                 start=True, stop=True, perf_mode=DR)
            srb = rmsp.tile([1, TOK], F32, tag="srb", name=f"srb_{lrep}")
            nc.scalar.copy(srb, srow)
            sct = psum.tile([128, 4], F32, tag="pst", name=f"sct_{lrep}")
            for t in range(4):
                nc.tensor.matmul(sct[:, t:t + 1], srb[:, t * 128:(t + 1) * 128],
                                 ones1, start=True, stop=True)
            rec = rbp.tile([128, 4], F32, tag="rec", name=f"rec_{lrep}")
            nc.vector.reciprocal(rec, sct)
            rrow = psum.tile([1, TOK], F32, tag="prow", name=f"rr_{lrep}")
            for t in range(4):
                nc.tensor.matmul(rrow[:, t * 128:(t + 1) * 128], rec[:, t:t + 1],
                                 identf, start=True, stop=True)
            rrb = rmsp.tile([1, TOK], F32, tag="rrb", name=f"rrb_{lrep}")
            nc.scalar.copy(rrb, rrow)
            nc.gpsimd.partition_broadcast(rcb, rrb)

            # ---- oT = (exp @ v).T via v-stationary DoubleRow; gate ----
            for e in range(EC):
                ops = psum.tile([128, TOK], F32, tag=ROT[e % 4])
                for i in range(2):
                    nc.tensor.matmul(ops[:, i * 256:(i + 1) * 256],
                                     vg8[:, 2 * i:2 * i + 2,
                                         e * 128:(e + 1) * 128],
                                     exp8[i], start=True, stop=True,
                                     perf_mode=DR)
                nc.vector.tensor_mul(ogT[e], uT[e], ops)

            # ---- out-projection (bf16) + attn-normalize + residual ----
            for hp in range(HC):
                dps = psum.tile([128, TOK], F32, tag=ROT[hp % 4])
                for e in range(EC):
                    nc.tensor.matmul(dps, wout[:, e, hp * 128:(hp + 1) * 128],
                                     ogT[e], start=(e == 0), stop=(e == EC - 1))
                tmp = rtmp.tile([128, TOK], F32, tag="rtm")
                nc.vector.tensor_mul(tmp, dps, rcb)
                nc.vector.tensor_add(hT[hp], hT[hp], tmp)
                tail_chunk(hp)

        # ---- final norm + unpatch (fnorm_w folded into upw on host) ----
        sumsq("fin")
        rt = rms_stats("fin")
        hfin = uT   # reuse dead uT tiles as bf16 normalized h
        for j in range(HC):
            # rb is (1/rms)/64; upw is scaled x64 on host to compensate
            nc.vector.tensor_mul(hfin[j], hT[j], rb)
        for mchunk, msz in ((0, 128), (1, 64)):
            ps = psum.tile([128, TOK], F32, tag=("pa", "pb")[mchunk])
            for j in range(HC):
                nc.tensor.matmul(ps[:msz, :],
                                 upw[:, j, mchunk * 128:mchunk * 128 + msz],
                                 hfin[j], start=(j == 0), stop=(j == HC - 1))
            osb = rtmp.tile([128, TOK], F32, tag="m1")
            nc.vector.tensor_copy(osb[:msz, :], ps[:msz, :])
            nc.sync.dma_start(d_out.ap()[mchunk * 128:mchunk * 128 + msz, :],
                              osb[:msz, :])

    nc.compile()
    return nc


_BUILD_CACHE = {}


def _get_nc(nl=NL, repeat=1):
    key = (nl, repeat)
    if key not in _BUILD_CACHE:
        _BUILD_CACHE[key] = _build(nl, repeat)
    return _BUILD_CACHE[key]


def _rope_tables():
    pos = np.arange(L)

    def sinemb(p, dim=64, base=1000.0):
        half = dim // 2
        freqs = np.exp(np.arange(half, dtype=np.float32)
                       * np.float32(-np.log(base) / (half - 1)))
        ang = p[:, None].astype(np.float32) * freqs[None, :]
        return np.concatenate([np.sin(ang), np.cos(ang)], axis=-1)

    w = IMG // P
    pe = np.concatenate([sinemb(pos // w), sinemb(pos % w)],
                        axis=-1).astype(np.float32)      # (256, 128)
    sinv = pe[:, :64].T                                  # (64, 256)
    cosv = pe[:, 64:].T
    COS = np.concatenate([cosv, cosv], axis=0)           # (128, 256)
    SINS = np.concatenate([-sinv, sinv], axis=0)
    COS2 = np.tile(COS, (1, 2))                          # (128, 512)
    SINS2 = np.tile(SINS, (1, 2))
    scale = np.float32(KD ** -0.5)
    return (np.ascontiguousarray(COS2 * scale).astype(BF),
            np.ascontiguousarray(SINS2 * scale).astype(BF),
            np.ascontiguousarray(COS2).astype(BF),
            np.ascontiguousarray(SINS2).astype(BF))


def _prep_weights(patch_W, t_emb, Wuv, Wout, gnorm, fnorm_w, unpatch_W, nl=NL):
    Wg = Wuv[:nl] * gnorm[:nl, :, None]                  # fold gnorm
    u = Wg[:, :, :E]
    v = Wg[:, :, E:2 * E]
    q = Wg[:, :, 2 * E:2 * E + KD]
    k = Wg[:, :, 2 * E + KD:]
    wuvp = np.concatenate([u, q, k, v], axis=2)          # (nl, 768, 3328)
    w8 = np.clip(SW * wuvp, -240.0, 240.0).astype(F8)
    wuv_h = np.ascontiguousarray(
        w8.reshape(nl, HC, 128, WUV_W).transpose(0, 2, 1, 3))
    wout_h = np.ascontiguousarray(
        Wout[:nl].reshape(nl, EC, 128, H).transpose(0, 2, 1, 3)).astype(BF)
    pw_pad = np.zeros((256, H), np.float32)
    pw_pad[:PD] = patch_W
    pw_h = np.ascontiguousarray(
        pw_pad.reshape(2, 128, H).transpose(1, 0, 2)).astype(BF)
    # final rb carries a 1/64 descale (shared Newton path); fold x64 here
    upw = SW * fnorm_w[:, None] * unpatch_W
    upw_h = np.ascontiguousarray(
        upw.reshape(HC, 128, PD).transpose(1, 0, 2)).astype(BF)
    return wuv_h, wout_h, pw_h, upw_h


def _patchify(xc):
    """(2,3,128,128) -> (512, 192) token-major patches."""
    g = IMG // P
    xp = xc.reshape(2, 3, g, P, g, P).transpose(0, 2, 4, 3, 5, 1)
    return np.ascontiguousarray(xp.reshape(2 * L, PD))


def _unpatchify(oT):
    """(192, 512) -> (2, 3, 128, 128)."""
    g = IMG // P
    out = np.empty((2, 3, IMG, IMG), np.float32)
    for i in range(2):
        h = oT[:, i * L:(i + 1) * L].T                   # (256, 192)
        out[i] = (h.reshape(g, g, P, P, 3)
                  .transpose(4, 0, 2, 1, 3).reshape(3, IMG, IMG))
    return out


def make_in_maps(x, t_idx, patch_W, t_emb, Wuv, Wout, gnorm, fnorm_w,
                 unpatch_W, nl=NL):
    x = np.asarray(x, np.float32)
    t_idx = np.asarray(t_idx).astype(np.int64)
    patch_W = np.asarray(patch_W, np.float32)
    t_emb = np.asarray(t_emb, np.float32)
    Wuv = np.asarray(Wuv, np.float32)
    Wout = np.asarray(Wout, np.float32)
    gnorm = np.asarray(gnorm, np.float32)
    fnorm_w = np.asarray(fnorm_w, np.float32)
    unpatch_W = np.asarray(unpatch_W, np.float32)

    wuv_h, wout_h, pw_h, upw_h = _prep_weights(
        patch_W, t_emb, Wuv, Wout, gnorm, fnorm_w, unpatch_W, nl)
    cqt, sqt, ckt, skt = _rope_tables()
    sperm = np.ascontiguousarray(np.roll(np.eye(128, dtype=np.float32),
                                         64, axis=0)).astype(BF)

    in_maps = []
    for c in range(NCORES):
        xc = x[2 * c:2 * c + 2]
        xp = _patchify(xc)                               # (512, 192)
        xpad = np.zeros((TOK, 256), np.float32)
        xpad[:, :PD] = xp
        xpt = np.ascontiguousarray(
            xpad.T.reshape(2, 128, TOK).transpose(1, 0, 2)).astype(BF)
        te = t_emb[t_idx[2 * c:2 * c + 2, 0]]            # (2, 768)
        tembT = np.ascontiguousarray(
            te.T.reshape(HC, 128, 2).transpose(1, 0, 2)).astype(np.float32)
        in_maps.append({
            "xpt": xpt, "temb": tembT, "pw": pw_h, "wuv": wuv_h,
            "wout": wout_h, "upw": upw_h, "cq": cqt, "sq": sqt,
            "ck": ckt, "sk": skt, "sperm": sperm,
        })
    return in_maps


def kernel(**inputs):
    from concourse.bass_utils import run_bass_kernel_spmd

    nc = _get_nc()
    in_maps = make_in_maps(**inputs)
    res = run_bass_kernel_spmd(nc, in_maps, core_ids=list(range(NCORES)))
    out = np.empty((B, 3, IMG, IMG), np.float32)
    for c in range(NCORES):
        out[2 * c:2 * c + 2] = _unpatchify(res.results[c]["outt"])
    return out


# revision 12
# speedup vs baseline: 1.5184x; 1.0958x over previous
"""GAU denoising transformer forward pass on 8 Trainium2 NeuronCores.

Data-parallel over batch (B=16 -> 2 images per core); identical NEFF per
core. Residual stream hT kept fp32 in SBUF, transposed (H on partitions x
512 tokens). Big GEMMs run fp8-e4m3, mostly with DoubleRow perf mode (two
K=128 chunks contracted per instruction):

  - wuv8 = e4m3(64 * Wuv * gnorm) streamed from HBM (half the bytes of
    bf16). All of u/v/q/k read h8raw = e4m3(h) (raw residual, so no GEMM
    ever waits on the rms stats); the 1/rms and the 1/64 fp8 descale are
    folded into the Newton-rsqrt output (rt/rb) and applied at psum
    evacuation, exactly like the bf16 baseline did.
  - attention is computed TRANSPOSED (scores [m,l] via k-stationary
    matmuls): softmax sums reduce over partitions with a ones-matmul, exp
    stays unnormalized in fp8, attn@v directly produces oT (feature on
    partitions) via DoubleRow, no PE transposes. The 1/sum(exp) row goes
    through the PE scatter -> DVE reciprocal -> PE gather shuffle and is
    folded in after the out-projection.
  - v's GEMM is activation-stationary with the stationary h8 pair reused
    across 3 moving blocks (amortizes DoubleRow's 256-column LDWEIGHTS).
  - out-projection stays bf16 (fp8 there costs too much accuracy).

ACT uses only Silu and Exp (plus copies): the per-layer table-set
sequence is [silu (v,u)] -> [exp (scores)], i.e. exactly 2 table loads
per layer, and the order is forced by real data dependencies. Squares
and fp8 casts of h run on gpsimd; rms rsqrt is a DVE Newton iteration on
a (128,4) shuffle of the sumsq row (no Ln/Rsqrt tables).
"""

import sys

for _p in ("/opt/trn_rl_repo",):
    if _p not in sys.path:
        sys.path.append(_p)

import numpy as np
import ml_dtypes

BF = ml_dtypes.bfloat16
F8 = ml_dtypes.float8_e4m3

IMG = 128
P = 8
H = 768
E = 1536
KD = 128          # key size
L = 256           # patches per image
PD = 192          # patch dim
NL = 24
B = 16
NCORES = 8
TOK = 512         # tokens per core (2 images x 256)
HC = H // 128     # 6 h-chunks
EC = E // 128     # 12 e-chunks
SW = 64.0         # fp8 weight scale
WUV_W = 2 * E + 2 * KD    # permuted wuv width: u | q | k | v
QO = E                    # q col offset
KO = E + KD
V0 = E + 2 * KD


def _build(nl=NL, repeat=1):
    """Build + compile the Bass module. Returns nc."""
    import concourse.tile as tile
    from concourse import bacc, mybir
    from concourse.masks import make_identity

    F32 = mybir.dt.float32
    BF16 = mybir.dt.bfloat16
    FP8 = mybir.dt.float8e4
    AF = mybir.ActivationFunctionType
    DR = mybir.MatmulPerfMode.DoubleRow

    nc = bacc.Bacc("TRN2", target_bir_lowering=False, debug=False,
                   num_devices=NCORES)

    d_xpt = nc.dram_tensor("xpt", [128, 2, TOK], BF16, kind="ExternalInput")
    d_temb = nc.dram_tensor("temb", [128, HC, 2], F32, kind="ExternalInput")
    d_pw = nc.dram_tensor("pw", [128, 2, H], BF16, kind="ExternalInput")
    d_wuv = nc.dram_tensor("wuv", [nl, 128, HC, WUV_W], FP8,
                           kind="ExternalInput")
    d_wout = nc.dram_tensor("wout", [nl, 128, EC, H], BF16,
                            kind="ExternalInput")
    d_upw = nc.dram_tensor("upw", [128, HC, PD], BF16, kind="ExternalInput")
    d_sperm = nc.dram_tensor("sperm", [128, 128], BF16, kind="ExternalInput")
    d_cq = nc.dram_tensor("cq", [128, TOK], BF16, kind="ExternalInput")
    d_sq = nc.dram_tensor("sq", [128, TOK], BF16, kind="ExternalInput")
    d_ck = nc.dram_tensor("ck", [128, TOK], BF16, kind="ExternalInput")
    d_sk = nc.dram_tensor("sk", [128, TOK], BF16, kind="ExternalInput")
    d_out = nc.dram_tensor("outt", [PD, TOK], F32, kind="ExternalOutput")

    from contextlib import ExitStack

    with tile.TileContext(nc) as tc, ExitStack() as ctx:
        pers = ctx.enter_context(tc.tile_pool(name="pers", bufs=1))
        wuvp = ctx.enter_context(tc.tile_pool(name="wuvp", bufs=2))
        woutp = ctx.enter_context(tc.tile_pool(name="woutp", bufs=2))
        rtmp = ctx.enter_context(tc.tile_pool(name="rtmp", bufs=2))
        utmp = ctx.enter_context(tc.tile_pool(name="utmp", bufs=3))
        rmsp = ctx.enter_context(tc.tile_pool(name="rmsp", bufs=2))
        rbp = ctx.enter_context(tc.tile_pool(name="rbp", bufs=2))

        psum = ctx.enter_context(tc.tile_pool(name="psum", bufs=1, space="PSUM"))

        # ---- persistent state + constants ----
        hT = [pers.tile([128, TOK], F32, name=f"hT{j}", tag=f"hT{j}")
              for j in range(HC)]
        h8raw = pers.tile([128, HC, TOK], FP8)
        hsq8 = pers.tile([128, HC, TOK], FP8)
        uT = [pers.tile([128, TOK], BF16, name=f"uT{e}", tag=f"uT{e}")
              for e in range(EC)]
        ogT = [pers.tile([128, TOK], BF16, name=f"ogT{e}", tag=f"ogT{e}")
               for e in range(EC)]
        vg8 = pers.tile([128, 4, E], FP8)
        exp8 = [pers.tile([128, 2, 256], FP8, name=f"exp8_{i}", tag=f"exp8_{i}")
                for i in range(2)]
        qp = pers.tile([128, TOK], BF16)
        kp = pers.tile([128, TOK], BF16)
        qsb = pers.tile([128, TOK], BF16)
        ksb = pers.tile([128, TOK], BF16)
        cq = pers.tile([128, TOK], BF16)
        sq = pers.tile([128, TOK], BF16)
        ck = pers.tile([128, TOK], BF16)
        sk = pers.tile([128, TOK], BF16)
        rb = pers.tile([128, TOK], F32)     # (1/rms)/64 broadcast
        rcb = pers.tile([128, TOK], F32)    # 1/sum(exp) broadcast
        temb = pers.tile([128, HC, 2], F32)
        xpt = pers.tile([128, 2, TOK], BF16)
        pw = pers.tile([128, 2, H], BF16)
        upw = pers.tile([128, HC, PD], BF16)
        sperm = pers.tile([128, 128], BF16)
        ones8 = pers.tile([128, 2, 16], FP8)
        twos8 = pers.tile([128, 2, 16], FP8)
        ones1 = pers.tile([1, 1], F32)
        identf = pers.tile([128, 128], F32)
        bln4 = pers.tile([128, 1], F32)     # -ln(4)

        nc.sync.dma_start(cq, d_cq.ap())
        nc.sync.dma_start(sq, d_sq.ap())
        nc.sync.dma_start(ck, d_ck.ap())
        nc.sync.dma_start(sk, d_sk.ap())
        nc.sync.dma_start(temb, d_temb.ap())
        nc.sync.dma_start(xpt, d_xpt.ap())
        nc.sync.dma_start(pw, d_pw.ap())
        nc.sync.dma_start(upw, d_upw.ap())
        nc.sync.dma_start(sperm, d_sperm.ap())
        nc.vector.memset(ones8, 1.0)
        nc.vector.memset(twos8, 2.0)
        nc.vector.memset(ones1, 1.0)
        nc.vector.memset(bln4, float(-np.log(4.0)))
        make_identity(nc, identf)

        state = {}

        def tail_chunk(hp):
            """After hT[hp] is final: fp8 cast (DVE, fast) + square (ACT,
            present in every table set). The sumsq matmuls are deferred to
            the next layer's start so the PE never waits here."""
            nc.vector.tensor_copy(h8raw[:, hp, :], hT[hp])
            nc.scalar.activation(hsq8[:, hp, :], hT[hp], AF.Square)

        def sumsq(name):
            """Accumulate the sumsq row (x2 via twos8) from hsq8."""
            ss = psum.tile([1, TOK], F32, tag="prow", name=f"ss_{name}")
            for t in range(3):
                nc.tensor.matmul(ss, twos8[:, :, 0:1],
                                 hsq8[:, 2 * t:2 * t + 2, :],
                                 start=(t == 0), stop=(t == 2),
                                 perf_mode=DR)
            state["ss"] = ss

        def rms_stats(name):
            """(1/rms)/64 from the sumsq row: (1,TOK)->(128,4) PE scatter,
            DVE Newton rsqrt, PE gather, gpsimd broadcast. Returns rt
            (128,4) with column t = (1/rms)/64 of token block t (the /64
            descales the fp8 weight scale)."""
            ssb = rmsp.tile([1, TOK], F32, tag="ssb", name=f"ssb_{name}")
            nc.scalar.copy(ssb, state["ss"])
            sst = psum.tile([128, 4], F32, tag="pst", name=f"sst_{name}")
            for t in range(4):
                nc.tensor.matmul(sst[:, t:t + 1], ssb[:, t * 128:(t + 1) * 128],
                                 ones1, start=True, stop=True)
            # m = 16*mean(h^2); clamp to the seed's convergence window;
            # rt = rsqrt(m)*4/64 = (1/rms)/64.
            m = rmsp.tile([128, 4], F32, tag="m", name=f"m_{name}")
            nc.vector.tensor_scalar(m, sst, 8.0 / H, None,
                                    mybir.AluOpType.mult)
            nc.vector.tensor_scalar(m, m, 0.15, 6.0,
                                    mybir.AluOpType.max,
                                    mybir.AluOpType.min)
            rt = rbp.tile([128, 4], F32, tag="rt", name=f"rt_{name}")
            t1 = rmsp.tile([128, 4], F32, tag="t1", name=f"t1_{name}")
            nc.vector.reciprocal(rt, m)
            nc.vector.tensor_scalar(rt, rt, 0.5, 0.5,
                                    mybir.AluOpType.mult,
                                    mybir.AluOpType.add)
            for _ in range(4):   # newton: y *= 1.5 - 0.5*m*y^2
                nc.vector.tensor_mul(t1, rt, rt)
                nc.vector.tensor_mul(t1, t1, m)
                nc.vector.tensor_scalar(t1, t1, -0.5, 1.5,
                                        mybir.AluOpType.mult,
                                        mybir.AluOpType.add)
                nc.vector.tensor_mul(rt, rt, t1)
            nc.vector.tensor_scalar_mul(rt, rt, 4.0 / SW)
            row = psum.tile([1, TOK], F32, tag="prow", name=f"row_{name}")
            for t in range(4):
                nc.tensor.matmul(row[:, t * 128:(t + 1) * 128], rt[:, t:t + 1],
                                 identf, start=True, stop=True)
            yrow = rmsp.tile([1, TOK], F32, tag="yrow", name=f"yrow_{name}")
            nc.scalar.copy(yrow, row)
            nc.gpsimd.partition_broadcast(rb, yrow)
            return rt

        # ---- patchify: hT = patch_W.T @ xp.T + temb ----
        for j in range(HC):
            ps = psum.tile([128, TOK], F32, tag=("pa", "pb")[j % 2])
            for c in range(2):
                nc.tensor.matmul(ps, pw[:, c, j * 128:(j + 1) * 128],
                                 xpt[:, c, :], start=(c == 0), stop=(c == 1))
            for i in range(2):
                nc.vector.tensor_scalar_add(
                    hT[j][:, i * 256:(i + 1) * 256],
                    ps[:, i * 256:(i + 1) * 256],
                    temb[:, j, i:i + 1])
        for j in range(HC):
            tail_chunk(j)

        ROT = ["pa", "pb", "pqs", "pks"]

        for lrep in range(nl * repeat):
            li = lrep % nl
            wuv = wuvp.tile([128, HC, WUV_W], FP8, tag="wuv")
            nc.sync.dma_start(wuv, d_wuv.ap()[li])
            wout = woutp.tile([128, EC, H], BF16, tag="wout")
            nc.sync.dma_start(wout, d_wout.ap()[li])

            # ---- q/k col-tiles (DoubleRow on raw h8) + swap via perm ----
            q_ps = psum.tile([128, TOK], F32, tag="pq")
            k_ps = psum.tile([128, TOK], F32, tag="pk")
            for ps, c0 in ((q_ps, QO), (k_ps, KO)):
                for t in range(3):
                    nc.tensor.matmul(ps, wuv[:, 2 * t:2 * t + 2, c0:c0 + 128],
                                     h8raw[:, 2 * t:2 * t + 2, :],
                                     start=(t == 0), stop=(t == 2),
                                     perf_mode=DR)
            nc.scalar.copy(qsb, q_ps)
            nc.scalar.copy(ksb, k_ps)
            qs_ps = psum.tile([128, TOK], F32, tag="pqs")
            nc.tensor.matmul(qs_ps, sperm, qsb, start=True, stop=True)
            ks_ps = psum.tile([128, TOK], F32, tag="pks")
            nc.tensor.matmul(ks_ps, sperm, ksb, start=True, stop=True)

            # ---- rope part 1 (no rb needed): frees the q/k psum banks
            #      early so the v-stream rotation can reuse them ----
            m1 = rtmp.tile([128, TOK], F32, tag="m1")
            m2 = rtmp.tile([128, TOK], F32, tag="m2")
            nc.vector.tensor_mul(m1, q_ps, cq)
            nc.vector.tensor_mul(m2, qs_ps, sq)
            nc.vector.tensor_add(m1, m1, m2)
            m3 = rtmp.tile([128, TOK], F32, tag="m3")
            m4 = rtmp.tile([128, TOK], F32, tag="m4")
            nc.vector.tensor_mul(m3, k_ps, ck)
            nc.vector.tensor_mul(m4, ks_ps, sk)
            nc.vector.tensor_add(m3, m3, m4)

            # rms stats for this layer (off the GEMM critical path: all
            # GEMMs read raw h8; rt/rb land at psum evacuation)
            sumsq(f"l{lrep}")
            rt = rms_stats(f"l{lrep}")

            # ---- rope part 2: x (1/rms)/64 ----
            nc.vector.tensor_mul(qp, m1, rb)
            nc.vector.tensor_mul(kp, m3, rb)

            # ---- v natural (h8 stationary reused over 3 moving blocks,
            #      DoubleRow); silu((1/rms)/64 * ps) with per-partition rt ----
            for tk in range(4):
                vps = [psum.tile([128, 512], F32, tag=ROT[(tk * 3 + ns) % 4],
                                 name=f"v{lrep}_{tk}_{ns}")
                       for ns in range(3)]
                for t in range(3):
                    for ns in range(3):
                        nc.tensor.matmul(
                            vps[ns],
                            h8raw[:, 2 * t:2 * t + 2, tk * 128:(tk + 1) * 128],
                            wuv[:, 2 * t:2 * t + 2,
                                V0 + ns * 512:V0 + (ns + 1) * 512],
                            start=(t == 0), stop=(t == 2), perf_mode=DR)
                for ns in range(3):
                    nc.scalar.activation(vg8[:, tk, ns * 512:(ns + 1) * 512],
                                         vps[ns], AF.Silu,
                                         scale=rt[:, tk:tk + 1])

            # ---- transposed scores + exp (fp8, unnormalized) ----
            for i in range(2):
                scp = psum.tile([128, 2, 256], F32, tag=("pq", "pk")[i])
                for mc in range(2):
                    nc.tensor.matmul(scp[:, mc, :],
                                     kp[:, i * 256 + mc * 128:
                                        i * 256 + (mc + 1) * 128],
                                     qp[:, i * 256:(i + 1) * 256],
                                     start=True, stop=True)
                nc.scalar.activation(exp8[i], scp, AF.Exp, scale=1.0,
                                     bias=bln4[:, 0:1])

            # ---- u col-tiles (weight-stationary, DoubleRow) ----
            for ct in range(EC):
                ps = psum.tile([128, TOK], F32, tag=ROT[ct % 4])
                for t in range(3):
                    nc.tensor.matmul(ps,
                                     wuv[:, 2 * t:2 * t + 2,
                                         ct * 128:(ct + 1) * 128],
                                     h8raw[:, 2 * t:2 * t + 2, :],
                                     start=(t == 0), stop=(t == 2),
                                     perf_mode=DR)
                ut = utmp.tile([128, TOK], BF16, tag="ut")
                nc.vector.tensor_mul(ut, ps, rb)
                nc.scalar.activation(uT[ct], ut, AF.Silu)

            # ---- softmax sums over m -> 1/sum row -> rcb ----
            srow = psum.tile([1, TOK], F32, tag="prow", name=f"sr_{lrep}")
            for i in range(2):
                nc.tensor.matmul(srow[:, i * 256:(i + 1) * 256],
                                 ones8[:, :, 0:1], exp8[i],
                                 start=True, stop=True, perf_mode=DR)
            srb = rmsp.tile([1, TOK], F32, tag="srb", name=f"srb_{lrep}")
            nc.scalar.copy(srb, srow)
            sct = psum.tile([128, 4], F32, tag="pst", name=f"sct_{lrep}")
            for t in range(4):
                nc.tensor.matmul(sct[:, t:t + 1], srb[:, t * 128:(t + 1) * 128],
                                 ones1, start=True, stop=True)
            rec = rbp.tile([128, 4], F32, tag="rec", name=f"rec_{lrep}")
            nc.vector.reciprocal(rec, sct)
            rrow = psum.tile([1, TOK], F32, tag="prow", name=f"rr_{lrep}")
            for t in range(4):
                nc.tensor.matmul(rrow[:, t * 128:(t + 1) * 128], rec[:, t:t + 1],
                                 identf, start=True, stop=True)
            rrb = rmsp.tile([1, TOK], F32, tag="rrb", name=f"rrb_{lrep}")
            nc.scalar.copy(rrb, rrow)
            nc.gpsimd.partition_broadcast(rcb, rrb)

            # ---- oT = (exp @ v).T via v-stationary DoubleRow; gate;
            #      out-projection (bf16) interleaved per e-chunk so the PE
            #      covers the gating DVE muls (pass A: hp 0-2 accumulate
            #      while attention streams; pass B: hp 3-5 afterwards) ----
            dpsA = [psum.tile([128, TOK], F32, tag=t, name=f"dA{lrep}_{i}")
                    for i, t in enumerate(("pq", "pk", "pqs"))]
            for e in range(EC):
                ops = psum.tile([128, TOK], F32, tag=("pa", "pb")[e % 2])
                for i in range(2):
                    nc.tensor.matmul(ops[:, i * 256:(i + 1) * 256],
                                     vg8[:, 2 * i:2 * i + 2,
                                         e * 128:(e + 1) * 128],
                                     exp8[i], start=True, stop=True,
                                     perf_mode=DR)
                nc.vector.tensor_mul(ogT[e], uT[e], ops)
                for hp in range(3):
                    nc.tensor.matmul(dpsA[hp],
                                     wout[:, e, hp * 128:(hp + 1) * 128],
                                     ogT[e], start=(e == 0), stop=(e == EC - 1))
            for hp in range(3):
                tmp = rtmp.tile([128, TOK], F32, tag="rtm")
                nc.vector.tensor_mul(tmp, dpsA[hp], rcb)
                nc.vector.tensor_add(hT[hp], hT[hp], tmp)
                tail_chunk(hp)
            for hp in range(3, HC):
                dps = psum.tile([128, TOK], F32,
                                tag=("pks", "pa", "pb")[hp - 3])
                for e in range(EC):
                    nc.tensor.matmul(dps, wout[:, e, hp * 128:(hp + 1) * 128],
                                     ogT[e], start=(e == 0), stop=(e == EC - 1))
                tmp = rtmp.tile([128, TOK], F32, tag="rtm")
                nc.vector.tensor_mul(tmp, dps, rcb)
                nc.vector.tensor_add(hT[hp], hT[hp], tmp)
                tail_chunk(hp)

        # ---- final norm + unpatch (fnorm_w folded into upw on host) ----
        sumsq("fin")
        rt = rms_stats("fin")
        hfin = uT   # reuse dead uT tiles as bf16 normalized h
        for j in range(HC):
            # rb is (1/rms)/64; upw is scaled x64 on host to compensate
            nc.vector.tensor_mul(hfin[j], hT[j], rb)
        for mchunk, msz in ((0, 128), (1, 64)):
            ps = psum.tile([128, TOK], F32, tag=("pa", "pb")[mchunk])
            for j in range(HC):
                nc.tensor.matmul(ps[:msz, :],
                                 upw[:, j, mchunk * 128:mchunk * 128 + msz],
                                 hfin[j], start=(j == 0), stop=(j == HC - 1))
            osb = rtmp.tile([128, TOK], F32, tag="m1")
            nc.vector.tensor_copy(osb[:msz, :], ps[:msz, :])
            nc.sync.dma_start(d_out.ap()[mchunk * 128:mchunk * 128 + msz, :],
                              osb[:msz, :])

    nc.compile()
    return nc


_BUILD_CACHE = {}


def _get_nc(nl=NL, repeat=1):
    key = (nl, repeat)
    if key not in _BUILD_CACHE:
        _BUILD_CACHE[key] = _build(nl, repeat)
    return _BUILD_CACHE[key]


def _rope_tables():
    pos = np.arange(L)

    def sinemb(p, dim=64, base=1000.0):
        half = dim // 2
        freqs = np.exp(np.arange(half, dtype=np.float32)
                       * np.float32(-np.log(base) / (half - 1)))
        ang = p[:, None].astype(np.float32) * freqs[None, :]
        return np.concatenate([np.sin(ang), np.cos(ang)], axis=-1)

    w = IMG // P
    pe = np.concatenate([sinemb(pos // w), sinemb(pos % w)],
                        axis=-1).astype(np.float32)      # (256, 128)
    sinv = pe[:, :64].T                                  # (64, 256)
    cosv = pe[:, 64:].T
    COS = np.concatenate([cosv, cosv], axis=0)           # (128, 256)
    SINS = np.concatenate([-sinv, sinv], axis=0)
    COS2 = np.tile(COS, (1, 2))                          # (128, 512)
    SINS2 = np.tile(SINS, (1, 2))
    scale = np.float32(KD ** -0.5)
    return (np.ascontiguousarray(COS2 * scale).astype(BF),
            np.ascontiguousarray(SINS2 * scale).astype(BF),
            np.ascontiguousarray(COS2).astype(BF),
            np.ascontiguousarray(SINS2).astype(BF))


def _prep_weights(patch_W, t_emb, Wuv, Wout, gnorm, fnorm_w, unpatch_W, nl=NL):
    Wg = Wuv[:nl] * gnorm[:nl, :, None]                  # fold gnorm
    u = Wg[:, :, :E]
    v = Wg[:, :, E:2 * E]
    q = Wg[:, :, 2 * E:2 * E + KD]
    k = Wg[:, :, 2 * E + KD:]
    wuvp = np.concatenate([u, q, k, v], axis=2)          # (nl, 768, 3328)
    w8 = np.clip(SW * wuvp, -240.0, 240.0).astype(F8)
    wuv_h = np.ascontiguousarray(
        w8.reshape(nl, HC, 128, WUV_W).transpose(0, 2, 1, 3))
    wout_h = np.ascontiguousarray(
        Wout[:nl].reshape(nl, EC, 128, H).transpose(0, 2, 1, 3)).astype(BF)
    pw_pad = np.zeros((256, H), np.float32)
    pw_pad[:PD] = patch_W
    pw_h = np.ascontiguousarray(
        pw_pad.reshape(2, 128, H).transpose(1, 0, 2)).astype(BF)
    # final rb carries a 1/64 descale (shared Newton path); fold x64 here
    upw = SW * fnorm_w[:, None] * unpatch_W
    upw_h = np.ascontiguousarray(
        upw.reshape(HC, 128, PD).transpose(1, 0, 2)).astype(BF)
    return wuv_h, wout_h, pw_h, upw_h


def _patchify(xc):
    """(2,3,128,128) -> (512, 192) token-major patches."""
    g = IMG // P
    xp = xc.reshape(2, 3, g, P, g, P).transpose(0, 2, 4, 3, 5, 1)
    return np.ascontiguousarray(xp.reshape(2 * L, PD))


def _unpatchify(oT):
    """(192, 512) -> (2, 3, 128, 128)."""
    g = IMG // P
    out = np.empty((2, 3, IMG, IMG), np.float32)
    for i in range(2):
        h = oT[:, i * L:(i + 1) * L].T                   # (256, 192)
        out[i] = (h.reshape(g, g, P, P, 3)
                  .transpose(4, 0, 2, 1, 3).reshape(3, IMG, IMG))
    return out


def make_in_maps(x, t_idx, patch_W, t_emb, Wuv, Wout, gnorm, fnorm_w,
                 unpatch_W, nl=NL):
    x = np.asarray(x, np.float32)
    t_idx = np.asarray(t_idx).astype(np.int64)
    patch_W = np.asarray(patch_W, np.float32)
    t_emb = np.asarray(t_emb, np.float32)
    Wuv = np.asarray(Wuv, np.float32)
    Wout = np.asarray(Wout, np.float32)
    gnorm = np.asarray(gnorm, np.float32)
    fnorm_w = np.asarray(fnorm_w, np.float32)
    unpatch_W = np.asarray(unpatch_W, np.float32)

    wuv_h, wout_h, pw_h, upw_h = _prep_weights(
        patch_W, t_emb, Wuv, Wout, gnorm, fnorm_w, unpatch_W, nl)
    cqt, sqt, ckt, skt = _rope_tables()
    sperm = np.ascontiguousarray(np.roll(np.eye(128, dtype=np.float32),
                                         64, axis=0)).astype(BF)

    in_maps = []
    for c in range(NCORES):
        xc = x[2 * c:2 * c + 2]
        xp = _patchify(xc)                               # (512, 192)
        xpad = np.zeros((TOK, 256), np.float32)
        xpad[:, :PD] = xp
        xpt = np.ascontiguousarray(
            xpad.T.reshape(2, 128, TOK).transpose(1, 0, 2)).astype(BF)
        te = t_emb[t_idx[2 * c:2 * c + 2, 0]]            # (2, 768)
        tembT = np.ascontiguousarray(
            te.T.reshape(HC, 128, 2).transpose(1, 0, 2)).astype(np.float32)
        in_maps.append({
            "xpt": xpt, "temb": tembT, "pw": pw_h, "wuv": wuv_h,
            "wout": wout_h, "upw": upw_h, "cq": cqt, "sq": sqt,
            "ck": ckt, "sk": skt, "sperm": sperm,
        })
    return in_maps


def kernel(**inputs):
    from concourse.bass_utils import run_bass_kernel_spmd

    nc = _get_nc()
    in_maps = make_in_maps(**inputs)
    res = run_bass_kernel_spmd(nc, in_maps, core_ids=list(range(NCORES)))
    out = np.empty((B, 3, IMG, IMG), np.float32)
    for c in range(NCORES):
        out[2 * c:2 * c + 2] = _unpatchify(res.results[c]["outt"])
    return out


# revision 13
# speedup vs baseline: 1.5768x; 1.0385x over previous
"""GAU denoising transformer forward pass on 8 Trainium2 NeuronCores.

Data-parallel over batch (B=16 -> 2 images per core); identical NEFF per
core. Residual stream hT kept fp32 in SBUF, transposed (H on partitions x
512 tokens). Big GEMMs run fp8-e4m3, mostly with DoubleRow perf mode (two
K=128 chunks contracted per instruction):

  - wuv8 = e4m3(64 * Wuv * gnorm) streamed from HBM (half the bytes of
    bf16). All of u/v/q/k read h8raw = e4m3(h) (raw residual, so no GEMM
    ever waits on the rms stats); the 1/rms and the 1/64 fp8 descale are
    folded into the Newton-rsqrt output (rt/rb) and applied at psum
    evacuation, exactly like the bf16 baseline did.
  - attention is computed TRANSPOSED (scores [m,l] via k-stationary
    matmuls): softmax sums reduce over partitions with a ones-matmul, exp
    stays unnormalized in fp8, attn@v directly produces oT (feature on
    partitions) via DoubleRow, no PE transposes. The 1/sum(exp) row goes
    through the PE scatter -> DVE reciprocal -> PE gather shuffle and is
    folded in after the out-projection.
  - v's GEMM is activation-stationary with the stationary h8 pair reused
    across 3 moving blocks (amortizes DoubleRow's 256-column LDWEIGHTS).
  - out-projection stays bf16 (fp8 there costs too much accuracy).

ACT uses only Silu and Exp (plus copies): the per-layer table-set
sequence is [silu (v,u)] -> [exp (scores)], i.e. exactly 2 table loads
per layer, and the order is forced by real data dependencies. Squares
and fp8 casts of h run on gpsimd; rms rsqrt is a DVE Newton iteration on
a (128,4) shuffle of the sumsq row (no Ln/Rsqrt tables).
"""

import sys

for _p in ("/opt/trn_rl_repo",):
    if _p not in sys.path:
        sys.path.append(_p)

import numpy as np
import ml_dtypes

BF = ml_dtypes.bfloat16
F8 = ml_dtypes.float8_e4m3

IMG = 128
P = 8
H = 768
E = 1536
KD = 128          # key size
L = 256           # patches per image
PD = 192          # patch dim
NL = 24
B = 16
NCORES = 8
TOK = 512         # tokens per core (2 images x 256)
HC = H // 128     # 6 h-chunks
EC = E // 128     # 12 e-chunks
SW = 64.0         # fp8 weight scale
WUV_W = 2 * E + 2 * KD    # permuted wuv width: u | q | k | v
QO = E                    # q col offset
KO = E + KD
V0 = E + 2 * KD


def _build(nl=NL, repeat=1):
    """Build + compile the Bass module. Returns nc."""
    import concourse.tile as tile
    from concourse import bacc, mybir
    from concourse.masks import make_identity

    F32 = mybir.dt.float32
    BF16 = mybir.dt.bfloat16
    FP8 = mybir.dt.float8e4
    AF = mybir.ActivationFunctionType
    DR = mybir.MatmulPerfMode.DoubleRow

    nc = bacc.Bacc("TRN2", target_bir_lowering=False, debug=False,
                   num_devices=NCORES)

    d_xpt = nc.dram_tensor("xpt", [128, 2, TOK], BF16, kind="ExternalInput")
    d_temb = nc.dram_tensor("temb", [128, HC, 2], F32, kind="ExternalInput")
    d_pw = nc.dram_tensor("pw", [128, 2, H], BF16, kind="ExternalInput")
    d_wuv = nc.dram_tensor("wuv", [nl, 128, HC, WUV_W], FP8,
                           kind="ExternalInput")
    d_wout = nc.dram_tensor("wout", [nl, 128, EC, H], BF16,
                            kind="ExternalInput")
    d_upw = nc.dram_tensor("upw", [128, HC, PD], BF16, kind="ExternalInput")
    d_sperm = nc.dram_tensor("sperm", [128, 128], BF16, kind="ExternalInput")
    d_cq = nc.dram_tensor("cq", [128, TOK], BF16, kind="ExternalInput")
    d_sq = nc.dram_tensor("sq", [128, TOK], BF16, kind="ExternalInput")
    d_ck = nc.dram_tensor("ck", [128, TOK], BF16, kind="ExternalInput")
    d_sk = nc.dram_tensor("sk", [128, TOK], BF16, kind="ExternalInput")
    d_out = nc.dram_tensor("outt", [PD, TOK], F32, kind="ExternalOutput")

    from contextlib import ExitStack

    with tile.TileContext(nc) as tc, ExitStack() as ctx:
        pers = ctx.enter_context(tc.tile_pool(name="pers", bufs=1))
        wuvp = ctx.enter_context(tc.tile_pool(name="wuvp", bufs=2))
        woutp = ctx.enter_context(tc.tile_pool(name="woutp", bufs=2))
        rtmp = ctx.enter_context(tc.tile_pool(name="rtmp", bufs=2))
        utmp = ctx.enter_context(tc.tile_pool(name="utmp", bufs=3))
        rmsp = ctx.enter_context(tc.tile_pool(name="rmsp", bufs=2))
        rbp = ctx.enter_context(tc.tile_pool(name="rbp", bufs=2))

        psum = ctx.enter_context(tc.tile_pool(name="psum", bufs=1, space="PSUM"))

        # ---- persistent state + constants ----
        hT = [pers.tile([128, TOK], F32, name=f"hT{j}", tag=f"hT{j}")
              for j in range(HC)]
        h8raw = pers.tile([128, HC, TOK], FP8)
        hsq8 = pers.tile([128, HC, TOK], FP8)
        uT = [pers.tile([128, TOK], BF16, name=f"uT{e}", tag=f"uT{e}")
              for e in range(EC)]
        ogT = [pers.tile([128, TOK], BF16, name=f"ogT{e}", tag=f"ogT{e}")
               for e in range(EC)]
        vg8 = pers.tile([128, 4, E], FP8)
        exp8 = [pers.tile([128, 2, 256], FP8, name=f"exp8_{i}", tag=f"exp8_{i}")
                for i in range(2)]
        qp = pers.tile([128, TOK], BF16)
        kp = pers.tile([128, TOK], BF16)
        qsb = pers.tile([128, TOK], BF16)
        ksb = pers.tile([128, TOK], BF16)
        cq = pers.tile([128, TOK], BF16)
        sq = pers.tile([128, TOK], BF16)
        ck = pers.tile([128, TOK], BF16)
        sk = pers.tile([128, TOK], BF16)
        rb = pers.tile([128, TOK], F32)     # (1/rms)/64 broadcast
        rcb = pers.tile([128, TOK], F32)    # 1/sum(exp) broadcast
        temb = pers.tile([128, HC, 2], F32)
        xpt = pers.tile([128, 2, TOK], BF16)
        pw = pers.tile([128, 2, H], BF16)
        upw = pers.tile([128, HC, PD], BF16)
        sperm = pers.tile([128, 128], BF16)
        ones8 = pers.tile([128, 2, 16], FP8)
        twos8 = pers.tile([128, 2, 16], FP8)
        ones1 = pers.tile([1, 1], F32)
        identf = pers.tile([128, 128], F32)
        bln4 = pers.tile([128, 1], F32)     # -ln(4)
        dumm = pers.tile([1, 1], F32)       # sqrt table-load hoist target

        nc.sync.dma_start(cq, d_cq.ap())
        nc.sync.dma_start(sq, d_sq.ap())
        nc.sync.dma_start(ck, d_ck.ap())
        nc.sync.dma_start(sk, d_sk.ap())
        nc.sync.dma_start(temb, d_temb.ap())
        nc.sync.dma_start(xpt, d_xpt.ap())
        nc.sync.dma_start(pw, d_pw.ap())
        nc.sync.dma_start(upw, d_upw.ap())
        nc.sync.dma_start(sperm, d_sperm.ap())
        nc.vector.memset(ones8, 1.0)
        nc.vector.memset(twos8, 2.0)
        nc.vector.memset(ones1, 1.0)
        nc.vector.memset(bln4, float(-np.log(4.0)))
        make_identity(nc, identf)

        state = {}

        def tail_chunk(hp):
            """After hT[hp] is final: fp8 cast (DVE, fast) + square (ACT,
            present in every table set). The sumsq matmuls are deferred to
            the next layer's start so the PE never waits here."""
            nc.vector.tensor_copy(h8raw[:, hp, :], hT[hp])
            nc.scalar.activation(hsq8[:, hp, :], hT[hp], AF.Square)

        def sumsq(name):
            """Accumulate the sumsq row (x2 via twos8) from hsq8."""
            ss = psum.tile([1, TOK], F32, tag="prow", name=f"ss_{name}")
            for t in range(3):
                nc.tensor.matmul(ss, twos8[:, :, 0:1],
                                 hsq8[:, 2 * t:2 * t + 2, :],
                                 start=(t == 0), stop=(t == 2),
                                 perf_mode=DR)
            state["ss"] = ss

        def rms_stats(name):
            """rt = (1/rms)/64 from the sumsq row via ACT Sqrt + DVE
            reciprocal: sqrt(ss * 64^2/(2H)) = 64*rms on a (1,TOK) row,
            PE-scatter to (128,4), one exact DVE reciprocal. Column t of
            rt = (1/rms)/64 for token block t."""
            sqrow = rmsp.tile([1, TOK], F32, tag="sqr", name=f"sqr_{name}")
            nc.scalar.activation(sqrow, state["ss"], AF.Sqrt,
                                 scale=SW * SW / (2.0 * H))
            sct = psum.tile([128, 4], F32, tag="pst", name=f"sst_{name}")
            for t in range(4):
                nc.tensor.matmul(sct[:, t:t + 1], sqrow[:, t * 128:(t + 1) * 128],
                                 ones1, start=True, stop=True)
            rt = rbp.tile([128, 4], F32, tag="rt", name=f"rt_{name}")
            nc.vector.reciprocal(rt, sct)
            return rt

        def rms_bcast(rt, name):
            """rb[128,TOK] broadcast of rt (PE gather -> copy -> gpsimd)."""
            row = psum.tile([1, TOK], F32, tag="prow", name=f"row_{name}")
            for t in range(4):
                nc.tensor.matmul(row[:, t * 128:(t + 1) * 128], rt[:, t:t + 1],
                                 identf, start=True, stop=True)
            yrow = rmsp.tile([1, TOK], F32, tag="yrow", name=f"yrow_{name}")
            nc.scalar.copy(yrow, row)
            nc.gpsimd.partition_broadcast(rb, yrow)

        # ---- patchify: hT = patch_W.T @ xp.T + temb ----
        for j in range(HC):
            ps = psum.tile([128, TOK], F32, tag=("pa", "pb")[j % 2])
            for c in range(2):
                nc.tensor.matmul(ps, pw[:, c, j * 128:(j + 1) * 128],
                                 xpt[:, c, :], start=(c == 0), stop=(c == 1))
            for i in range(2):
                nc.vector.tensor_scalar_add(
                    hT[j][:, i * 256:(i + 1) * 256],
                    ps[:, i * 256:(i + 1) * 256],
                    temb[:, j, i:i + 1])
        for j in range(HC):
            tail_chunk(j)

        ROT = ["pa", "pb", "pqs", "pks"]

        for lrep in range(nl * repeat):
            li = lrep % nl
            wuv = wuvp.tile([128, HC, WUV_W], FP8, tag="wuv")
            nc.sync.dma_start(wuv, d_wuv.ap()[li])
            wout = woutp.tile([128, EC, H], BF16, tag="wout")
            nc.sync.dma_start(wout, d_wout.ap()[li])

            # ---- q/k col-tiles (DoubleRow on raw h8) + swap via perm ----
            q_ps = psum.tile([128, TOK], F32, tag="pq")
            k_ps = psum.tile([128, TOK], F32, tag="pk")
            for ps, c0 in ((q_ps, QO), (k_ps, KO)):
                for t in range(3):
                    nc.tensor.matmul(ps, wuv[:, 2 * t:2 * t + 2, c0:c0 + 128],
                                     h8raw[:, 2 * t:2 * t + 2, :],
                                     start=(t == 0), stop=(t == 2),
                                     perf_mode=DR)
            nc.scalar.copy(qsb, q_ps)
            nc.scalar.copy(ksb, k_ps)
            qs_ps = psum.tile([128, TOK], F32, tag="pqs")
            nc.tensor.matmul(qs_ps, sperm, qsb, start=True, stop=True)
            ks_ps = psum.tile([128, TOK], F32, tag="pks")
            nc.tensor.matmul(ks_ps, sperm, ksb, start=True, stop=True)

            # sumsq + sqrt-based stats (all GEMMs read raw h8, so this
            # sits off the GEMM critical path; rt lands at psum evacuation)
            sumsq(f"l{lrep}")

            # ---- rope part 1 (no rb needed): frees the q/k psum banks
            #      early so the v-stream rotation can reuse them ----
            m1 = rtmp.tile([128, TOK], F32, tag="m1")
            m2 = rtmp.tile([128, TOK], F32, tag="m2")
            nc.vector.tensor_mul(m1, q_ps, cq)
            nc.vector.tensor_mul(m2, qs_ps, sq)
            nc.vector.tensor_add(m1, m1, m2)
            m3 = rtmp.tile([128, TOK], F32, tag="m3")
            m4 = rtmp.tile([128, TOK], F32, tag="m4")
            nc.vector.tensor_mul(m3, k_ps, ck)
            nc.vector.tensor_mul(m4, ks_ps, sk)
            nc.vector.tensor_add(m3, m3, m4)

            rt = rms_stats(f"l{lrep}")

            def v_chunk(tk):
                vps = [psum.tile([128, 512], F32, tag=ROT[(tk * 3 + ns) % 4],
                                 name=f"v{lrep}_{tk}_{ns}")
                       for ns in range(3)]
                for t in range(3):
                    for ns in range(3):
                        nc.tensor.matmul(
                            vps[ns],
                            h8raw[:, 2 * t:2 * t + 2, tk * 128:(tk + 1) * 128],
                            wuv[:, 2 * t:2 * t + 2,
                                V0 + ns * 512:V0 + (ns + 1) * 512],
                            start=(t == 0), stop=(t == 2), perf_mode=DR)
                for ns in range(3):
                    nc.scalar.activation(vg8[:, tk, ns * 512:(ns + 1) * 512],
                                         vps[ns], AF.Silu,
                                         scale=rt[:, tk:tk + 1])

            # ---- v natural (h8 stationary reused over 3 moving blocks,
            #      DoubleRow): first block fills the PE while the rt
            #      reciprocal finishes, then the rb gather/broadcast ----
            v_chunk(0)
            rms_bcast(rt, f"l{lrep}")
            for tk in range(1, 4):
                v_chunk(tk)

            # ---- rope part 2: x (1/rms)/64 ----
            nc.vector.tensor_mul(qp, m1, rb)
            nc.vector.tensor_mul(kp, m3, rb)

            # ---- transposed scores + exp (fp8, unnormalized) ----
            for i in range(2):
                scp = psum.tile([128, 2, 256], F32, tag=("pq", "pk")[i])
                for mc in range(2):
                    nc.tensor.matmul(scp[:, mc, :],
                                     kp[:, i * 256 + mc * 128:
                                        i * 256 + (mc + 1) * 128],
                                     qp[:, i * 256:(i + 1) * 256],
                                     start=True, stop=True)
                nc.scalar.activation(exp8[i], scp, AF.Exp, scale=1.0,
                                     bias=bln4[:, 0:1])

            # ---- u col-tiles (weight-stationary, DoubleRow) ----
            for ct in range(EC):
                ps = psum.tile([128, TOK], F32, tag=ROT[ct % 4])
                for t in range(3):
                    nc.tensor.matmul(ps,
                                     wuv[:, 2 * t:2 * t + 2,
                                         ct * 128:(ct + 1) * 128],
                                     h8raw[:, 2 * t:2 * t + 2, :],
                                     start=(t == 0), stop=(t == 2),
                                     perf_mode=DR)
                ut = utmp.tile([128, TOK], BF16, tag="ut")
                nc.vector.tensor_mul(ut, ps, rb)
                nc.scalar.activation(uT[ct], ut, AF.Silu)

            # ---- softmax sums over m -> 1/sum row -> rcb ----
            srow = psum.tile([1, TOK], F32, tag="prow", name=f"sr_{lrep}")
            for i in range(2):
                nc.tensor.matmul(srow[:, i * 256:(i + 1) * 256],
                                 ones8[:, :, 0:1], exp8[i],
                                 start=True, stop=True, perf_mode=DR)
            srb = rmsp.tile([1, TOK], F32, tag="srb", name=f"srb_{lrep}")
            nc.scalar.copy(srb, srow)
            sct = psum.tile([128, 4], F32, tag="pst", name=f"sct_{lrep}")
            for t in range(4):
                nc.tensor.matmul(sct[:, t:t + 1], srb[:, t * 128:(t + 1) * 128],
                                 ones1, start=True, stop=True)
            rec = rbp.tile([128, 4], F32, tag="rec", name=f"rec_{lrep}")
            nc.vector.reciprocal(rec, sct)

            # ---- oT = (exp @ v).T via v-stationary DoubleRow; gate;
            #      out-projection (bf16) interleaved per e-chunk so the PE
            #      covers the gating DVE muls (pass A: hp 0-2 accumulate
            #      while attention streams; pass B: hp 3-5 afterwards) ----
            dpsA = [psum.tile([128, TOK], F32, tag=t, name=f"dA{lrep}_{i}")
                    for i, t in enumerate(("pq", "pk", "pqs"))]
            for e in range(EC):
                ops = psum.tile([128, TOK], F32, tag=("pa", "pb")[e % 2])
                for i in range(2):
                    nc.tensor.matmul(ops[:, i * 256:(i + 1) * 256],
                                     vg8[:, 2 * i:2 * i + 2,
                                         e * 128:(e + 1) * 128],
                                     exp8[i], start=True, stop=True,
                                     perf_mode=DR)
                nc.vector.tensor_mul(ogT[e], uT[e], ops)
                for hp in range(3):
                    nc.tensor.matmul(dpsA[hp],
                                     wout[:, e, hp * 128:(hp + 1) * 128],
                                     ogT[e], start=(e == 0), stop=(e == EC - 1))
                if e == 0:
                    rrow = psum.tile([1, TOK], F32, tag="prow",
                                     name=f"rr_{lrep}")
                    for t in range(4):
                        nc.tensor.matmul(rrow[:, t * 128:(t + 1) * 128],
                                         rec[:, t:t + 1], identf,
                                         start=True, stop=True)
                    rrb = rmsp.tile([1, TOK], F32, tag="rrb",
                                    name=f"rrb_{lrep}")
                    nc.scalar.copy(rrb, rrow)
                    nc.gpsimd.partition_broadcast(rcb, rrb)
                    # hoist the sqrt table load off next layer's stats chain
                    nc.scalar.activation(dumm, ones1, AF.Sqrt)
            for hp in range(3):
                tmp = rtmp.tile([128, TOK], F32, tag="rtm")
                nc.vector.tensor_mul(tmp, dpsA[hp], rcb)
                nc.vector.tensor_add(hT[hp], hT[hp], tmp)
                tail_chunk(hp)
            for hp in range(3, HC):
                dps = psum.tile([128, TOK], F32,
                                tag=("pks", "pa", "pb")[hp - 3])
                for e in range(EC):
                    nc.tensor.matmul(dps, wout[:, e, hp * 128:(hp + 1) * 128],
                                     ogT[e], start=(e == 0), stop=(e == EC - 1))
                tmp = rtmp.tile([128, TOK], F32, tag="rtm")
                nc.vector.tensor_mul(tmp, dps, rcb)
                nc.vector.tensor_add(hT[hp], hT[hp], tmp)
                tail_chunk(hp)

        # ---- final norm + unpatch (fnorm_w folded into upw on host) ----
        sumsq("fin")
        rt = rms_stats("fin")
        rms_bcast(rt, "fin")
        hfin = uT   # reuse dead uT tiles as bf16 normalized h
        for j in range(HC):
            # rb is (1/rms)/64; upw is scaled x64 on host to compensate
            nc.vector.tensor_mul(hfin[j], hT[j], rb)
        for mchunk, msz in ((0, 128), (1, 64)):
            ps = psum.tile([128, TOK], F32, tag=("pa", "pb")[mchunk])
            for j in range(HC):
                nc.tensor.matmul(ps[:msz, :],
                                 upw[:, j, mchunk * 128:mchunk * 128 + msz],
                                 hfin[j], start=(j == 0), stop=(j == HC - 1))
            osb = rtmp.tile([128, TOK], F32, tag="m1")
            nc.vector.tensor_copy(osb[:msz, :], ps[:msz, :])
            nc.sync.dma_start(d_out.ap()[mchunk * 128:mchunk * 128 + msz, :],
                              osb[:msz, :])

    nc.compile()
    return nc


_BUILD_CACHE = {}


def _get_nc(nl=NL, repeat=1):
    key = (nl, repeat)
    if key not in _BUILD_CACHE:
        _BUILD_CACHE[key] = _build(nl, repeat)
    return _BUILD_CACHE[key]


def _rope_tables():
    pos = np.arange(L)

    def sinemb(p, dim=64, base=1000.0):
        half = dim // 2
        freqs = np.exp(np.arange(half, dtype=np.float32)
                       * np.float32(-np.log(base) / (half - 1)))
        ang = p[:, None].astype(np.float32) * freqs[None, :]
        return np.concatenate([np.sin(ang), np.cos(ang)], axis=-1)

    w = IMG // P
    pe = np.concatenate([sinemb(pos // w), sinemb(pos % w)],
                        axis=-1).astype(np.float32)      # (256, 128)
    sinv = pe[:, :64].T                                  # (64, 256)
    cosv = pe[:, 64:].T
    COS = np.concatenate([cosv, cosv], axis=0)           # (128, 256)
    SINS = np.concatenate([-sinv, sinv], axis=0)
    COS2 = np.tile(COS, (1, 2))                          # (128, 512)
    SINS2 = np.tile(SINS, (1, 2))
    scale = np.float32(KD ** -0.5)
    return (np.ascontiguousarray(COS2 * scale).astype(BF),
            np.ascontiguousarray(SINS2 * scale).astype(BF),
            np.ascontiguousarray(COS2).astype(BF),
            np.ascontiguousarray(SINS2).astype(BF))


def _prep_weights(patch_W, t_emb, Wuv, Wout, gnorm, fnorm_w, unpatch_W, nl=NL):
    Wg = Wuv[:nl] * gnorm[:nl, :, None]                  # fold gnorm
    u = Wg[:, :, :E]
    v = Wg[:, :, E:2 * E]
    q = Wg[:, :, 2 * E:2 * E + KD]
    k = Wg[:, :, 2 * E + KD:]
    wuvp = np.concatenate([u, q, k, v], axis=2)          # (nl, 768, 3328)
    w8 = np.clip(SW * wuvp, -240.0, 240.0).astype(F8)
    wuv_h = np.ascontiguousarray(
        w8.reshape(nl, HC, 128, WUV_W).transpose(0, 2, 1, 3))
    wout_h = np.ascontiguousarray(
        Wout[:nl].reshape(nl, EC, 128, H).transpose(0, 2, 1, 3)).astype(BF)
    pw_pad = np.zeros((256, H), np.float32)
    pw_pad[:PD] = patch_W
    pw_h = np.ascontiguousarray(
        pw_pad.reshape(2, 128, H).transpose(1, 0, 2)).astype(BF)
    # final rb carries a 1/64 descale (shared Newton path); fold x64 here
    upw = SW * fnorm_w[:, None] * unpatch_W
    upw_h = np.ascontiguousarray(
        upw.reshape(HC, 128, PD).transpose(1, 0, 2)).astype(BF)
    return wuv_h, wout_h, pw_h, upw_h


def _patchify(xc):
    """(2,3,128,128) -> (512, 192) token-major patches."""
    g = IMG // P
    xp = xc.reshape(2, 3, g, P, g, P).transpose(0, 2, 4, 3, 5, 1)
    return np.ascontiguousarray(xp.reshape(2 * L, PD))


def _unpatchify(oT):
    """(192, 512) -> (2, 3, 128, 128)."""
    g = IMG // P
    out = np.empty((2, 3, IMG, IMG), np.float32)
    for i in range(2):
        h = oT[:, i * L:(i + 1) * L].T                   # (256, 192)
        out[i] = (h.reshape(g, g, P, P, 3)
                  .transpose(4, 0, 2, 1, 3).reshape(3, IMG, IMG))
    return out


def make_in_maps(x, t_idx, patch_W, t_emb, Wuv, Wout, gnorm, fnorm_w,
                 unpatch_W, nl=NL):
    x = np.asarray(x, np.float32)
    t_idx = np.asarray(t_idx).astype(np.int64)
    patch_W = np.asarray(patch_W, np.float32)
    t_emb = np.asarray(t_emb, np.float32)
    Wuv = np.asarray(Wuv, np.float32)
    Wout = np.asarray(Wout, np.float32)
    gnorm = np.asarray(gnorm, np.float32)
    fnorm_w = np.asarray(fnorm_w, np.float32)
    unpatch_W = np.asarray(unpatch_W, np.float32)

    wuv_h, wout_h, pw_h, upw_h = _prep_weights(
        patch_W, t_emb, Wuv, Wout, gnorm, fnorm_w, unpatch_W, nl)
    cqt, sqt, ckt, skt = _rope_tables()
    sperm = np.ascontiguousarray(np.roll(np.eye(128, dtype=np.float32),
                                         64, axis=0)).astype(BF)

    in_maps = []
    for c in range(NCORES):
        xc = x[2 * c:2 * c + 2]
        xp = _patchify(xc)                               # (512, 192)
        xpad = np.zeros((TOK, 256), np.float32)
        xpad[:, :PD] = xp
        xpt = np.ascontiguousarray(
            xpad.T.reshape(2, 128, TOK).transpose(1, 0, 2)).astype(BF)
        te = t_emb[t_idx[2 * c:2 * c + 2, 0]]            # (2, 768)
        tembT = np.ascontiguousarray(
            te.T.reshape(HC, 128, 2).transpose(1, 0, 2)).astype(np.float32)
        in_maps.append({
            "xpt": xpt, "temb": tembT, "pw": pw_h, "wuv": wuv_h,
            "wout": wout_h, "upw": upw_h, "cq": cqt, "sq": sqt,
            "ck": ckt, "sk": skt, "sperm": sperm,
        })
    return in_maps


def kernel(**inputs):
    from concourse.bass_utils import run_bass_kernel_spmd

    nc = _get_nc()
    in_maps = make_in_maps(**inputs)
    res = run_bass_kernel_spmd(nc, in_maps, core_ids=list(range(NCORES)))
    out = np.empty((B, 3, IMG, IMG), np.float32)
    for c in range(NCORES):
        out[2 * c:2 * c + 2] = _unpatchify(res.results[c]["outt"])
    return out
